# revision 1
# baseline (speedup 1.0000x reference)
"""nn_DecoderBlock Trainium2 kernel — 8 NeuronCores, token-sharded.

Self-contained: builds a Bass/Tile SPMD program (one program, all 8
cores; per-core differences are input data), runs it via
run_bass_kernel_spmd, reassembles the full output on the host.
"""



import math
from contextlib import ExitStack

import numpy as np
import ml_dtypes

import concourse.bass as bass
import concourse.mybir as mybir
from concourse.tile import TileContext
from concourse.masks import make_identity

try:
    from tile_patch import split_excess_waits
except ImportError:  # self-contained kernel.py defines it later in-file
    pass

F32 = mybir.dt.float32
BF16 = mybir.dt.bfloat16
AF = mybir.ActivationFunctionType
ALU = mybir.AluOpType
AX = mybir.AxisListType

NEG = -1.0e9
CORES = 8
GPC = 4


def full_cfg():
    return dict(B=2, T=2048, D=2048, H=16, DFF=4096)


def small_cfg():
    return dict(B=2, T=512, D=512, H=4, DFF=1024)


def derived(cfg):
    B, T, D, H, DFF = cfg["B"], cfg["T"], cfg["D"], cfg["H"], cfg["DFF"]
    HD = D // H
    assert HD == 128
    TOK = B * T // CORES
    assert T // GPC == TOK and TOK % 128 == 0
    return dict(HD=HD, TOK=TOK, NT=TOK // 128, KD=D // 128, KF=DFF // 128,
                NKB=T // 128)


def build(nc: bass.Bass, cfg):
    B, T, D, H, DFF = cfg["B"], cfg["T"], cfg["D"], cfg["H"], cfg["DFF"]
    dv = derived(cfg)
    TOK, NT, KD, KF, NKB = dv["TOK"], dv["NT"], dv["KD"], dv["KF"], dv["NKB"]
    DCH = min(512, D)
    NDC = D // DCH
    RMS_EPS = float(np.finfo(np.float32).eps)
    LN_EPS = 1e-5
    DT = D * TOK

    x_in = nc.declare_dram_parameter("x", [TOK, D], F32, isOutput=False)
    wq = nc.declare_dram_parameter("wq", [D, D], BF16, isOutput=False)
    wk = nc.declare_dram_parameter("wk", [D, D], BF16, isOutput=False)
    wv = nc.declare_dram_parameter("wv", [D, D], BF16, isOutput=False)
    wo = nc.declare_dram_parameter("wo", [D, D], BF16, isOutput=False)
    w1 = nc.declare_dram_parameter("w1", [D, DFF], BF16, isOutput=False)
    wg1 = nc.declare_dram_parameter("wg1", [DFF, DFF], BF16, isOutput=False)
    wg2 = nc.declare_dram_parameter("wg2", [DFF, DFF], BF16, isOutput=False)
    w2 = nc.declare_dram_parameter("w2", [DFF, D], BF16, isOutput=False)
    bqc_d = nc.declare_dram_parameter("bqc", [D], F32, isOutput=False)
    bkp_d = nc.declare_dram_parameter("bkp", [D], F32, isOutput=False)
    b1_d = nc.declare_dram_parameter("b1p", [DFF], F32, isOutput=False)
    bg1_d = nc.declare_dram_parameter("bg1", [DFF], F32, isOutput=False)
    bg2_d = nc.declare_dram_parameter("bg2", [DFF], F32, isOutput=False)
    bo_rep_d = nc.declare_dram_parameter("bo_rep", [128, D], F32, isOutput=False)
    b2_rep_d = nc.declare_dram_parameter("b2_rep", [128, D], F32, isOutput=False)
    cos_d = nc.declare_dram_parameter("cosT", [128, TOK], F32, isOutput=False)
    sin_d = nc.declare_dram_parameter("sinT", [128, TOK], F32, isOutput=False)
    keybias_d = nc.declare_dram_parameter("keybias", [T], F32, isOutput=False)
    kbown_d = nc.declare_dram_parameter("keybias_own", [TOK], F32, isOutput=False)
    tri_d = nc.declare_dram_parameter("triT", [128, 128], F32, isOutput=False)
    out_d = nc.declare_dram_parameter("out", [TOK, D], F32, isOutput=True)

    with TileContext(nc) as tc, ExitStack() as top:
        constp = top.enter_context(tc.tile_pool(name="constp", bufs=1))
        dramp = top.enter_context(tc.tile_pool(name="dramp", bufs=1, space="DRAM"))
        wsp = top.enter_context(tc.tile_pool(name="wsp", bufs=16))
        x2p = top.enter_context(tc.tile_pool(name="x2p", bufs=1))

        # ---- constants
        ident = constp.tile([128, 128], BF16, name="ident")
        make_identity(nc, ident[:])
        ones_col = constp.tile([128, 1], BF16, name="ones_col")
        nc.vector.memset(ones_col[:], 1.0)
        ones_row = constp.tile([1, 128], F32, name="ones_row")
        nc.vector.memset(ones_row[:], 1.0)
        tri = constp.tile([128, 128], F32, name="tri")
        nc.sync.dma_start(tri[:], tri_d[:])
        cosT = constp.tile([128, TOK], F32, name="cosT")
        sinT = constp.tile([128, TOK], F32, name="sinT")
        nc.sync.dma_start(cosT[:], cos_d[:])
        nc.sync.dma_start(sinT[:], sin_d[:])
        kb_bias = constp.tile([128, NKB], F32, name="kb_bias")
        nc.sync.dma_start(kb_bias[:], keybias_d[:].rearrange("(n p) -> p n", p=128))
        kbo_bias = constp.tile([128, NT], F32, name="kbo_bias")
        nc.sync.dma_start(kbo_bias[:], kbown_d[:].rearrange("(n p) -> p n", p=128))
        bqc = constp.tile([128, KD], F32, name="bqc")
        nc.sync.dma_start(bqc[:], bqc_d[:].rearrange("(n p) -> p n", p=128))
        bkp = constp.tile([128, KD], F32, name="bkp")
        nc.sync.dma_start(bkp[:], bkp_d[:].rearrange("(n p) -> p n", p=128))
        b1t = constp.tile([128, KF], F32, name="b1t")
        nc.sync.dma_start(b1t[:], b1_d[:].rearrange("(n p) -> p n", p=128))
        bg1t = constp.tile([128, KF], F32, name="bg1t")
        nc.sync.dma_start(bg1t[:], bg1_d[:].rearrange("(n p) -> p n", p=128))
        bg2t = constp.tile([128, KF], F32, name="bg2t")
        nc.sync.dma_start(bg2t[:], bg2_d[:].rearrange("(n p) -> p n", p=128))
        bo_rep = constp.tile([128, D], F32, name="bo_rep")
        nc.sync.dma_start(bo_rep[:], bo_rep_d[:])
        b2_rep = constp.tile([128, D], F32, name="b2_rep")
        nc.sync.dma_start(b2_rep[:], b2_rep_d[:])

        snd_k = dramp.tile([DT], BF16, name="snd_k")
        snd_v = dramp.tile([DT], BF16, name="snd_v")
        gat_k = dramp.tile([GPC, DT], BF16, name="gat_k")
        gat_v = dramp.tile([GPC, DT], BF16, name="gat_v")

        x2_t = [x2p.tile([128, D], F32, name=f"x2_{t}") for t in range(NT)]
        sums_x2 = [x2p.tile([128, 1], F32, name=f"sx2_{t}") for t in range(NT)]

        with tc.tile_pool(name="ctxp", bufs=1) as ctxp:
            ctxT = [ctxp.tile([128, TOK], BF16, name=f"ctxT_{h}")
                    for h in range(H)]

            with tc.tile_pool(name="hTp", bufs=1) as hTp:
                hT = [hTp.tile([128, TOK], BF16, name=f"hT_{k}")
                      for k in range(KD)]

                # ===== phase 1: RMSNorm + transpose -> hT
                with tc.tile_pool(name="ph1w", bufs=2) as ph1w, \
                     tc.tile_pool(name="ps1", bufs=4, space="PSUM") as ps1:
                    for t in range(NT):
                        xt = ph1w.tile([128, D], F32, name="xt", tag="xt")
                        nc.sync.dma_start(xt[:], x_in[t * 128:(t + 1) * 128, :])
                        ss = ph1w.tile([128, NDC], F32, name="ss", tag="ss")
                        sq = ph1w.tile([128, DCH], F32, name="sq", tag="sq")
                        for c in range(NDC):
                            nc.scalar.activation(
                                sq[:], xt[:, c * DCH:(c + 1) * DCH], AF.Square,
                                accum_out=ss[:, c:c + 1])
                        ssum = ph1w.tile([128, 1], F32, name="ssum", tag="ssum")
                        nc.vector.tensor_reduce(ssum[:], ss[:], axis=AX.X,
                                                op=ALU.add)
                        nc.vector.tensor_scalar(
                            ssum[:], ssum[:], 1.0 / D, RMS_EPS,
                            op0=ALU.mult, op1=ALU.add)
                        nc.scalar.sqrt(ssum[:], ssum[:])
                        rs = ph1w.tile([128, 1], F32, name="rs", tag="rs")
                        nc.vector.reciprocal(rs[:], ssum[:])
                        hn = ph1w.tile([128, D], BF16, name="hn",
                                       tag="hn", bufs=2)
                        nc.scalar.activation(hn[:], xt[:], AF.Copy, scale=rs[:])
                        for k in range(KD):
                            tp = ps1.tile([128, 128], BF16, name="tp", tag="tp")
                            nc.tensor.transpose(
                                tp[:], hn[:, k * 128:(k + 1) * 128], ident[:])
                            nc.scalar.copy(hT[k][:, t * 128:(t + 1) * 128],
                                           tp[:])

                with tc.tile_pool(name="qkvp", bufs=1) as qkvp:
                    qrT = [qkvp.tile([128, TOK], BF16, name=f"qrT_{k}")
                           for k in range(KD)]
                    krT = [qkvp.tile([128, TOK], BF16, name=f"krT_{k}")
                           for k in range(KD)]
                    vtok = [qkvp.tile([128, D], BF16, name=f"vtok_{t}")
                            for t in range(NT)]

                    # ===== phase 2: projections + rope + send + gather
                    with tc.tile_pool(name="ph2w", bufs=4) as ph2w, \
                         tc.tile_pool(name="ps2", bufs=2, space="PSUM") as ps2:

                        def rope(dst, src):
                            # walrus: SB+SB tensor_tensor operands must share
                            # base partition -> cos/sin are replicated on both
                            # halves and tmps live at base 0
                            t1 = ph2w.tile([64, TOK], F32, name="rp1", tag="rp1")
                            t2 = ph2w.tile([64, TOK], F32, name="rp2", tag="rp2")
                            t3 = ph2w.tile([64, TOK], F32, name="rp3", tag="rp3")
                            t4 = ph2w.tile([64, TOK], F32, name="rp4", tag="rp4")
                            nc.vector.tensor_mul(t1[:], src[0:64, :], cosT[0:64, :])
                            nc.vector.tensor_mul(t2[:], src[64:128, :], sinT[64:128, :])
                            nc.vector.tensor_sub(dst[0:64, :], t1[:], t2[:])
                            nc.vector.tensor_mul(t3[:], src[0:64, :], sinT[0:64, :])
                            nc.vector.tensor_mul(t4[:], src[64:128, :], cosT[64:128, :])
                            nc.vector.tensor_add(dst[64:128, :], t3[:], t4[:])

                        qscale = 1.0 / math.sqrt(128.0)

                        def proj_fmajor(wten, bias_t, scale_, dstl, send):
                            for mb in range(KD // 4):
                                psl = [ps2.tile([128, DCH], F32, name=f"mm{m}",
                                                tag=f"mm{m}") for m in range(4)]
                                for k in range(KD):
                                    wt = wsp.tile([128, 512], BF16, name="wt",
                                                  tag="w")
                                    nc.sync.dma_start(
                                        wt[:], wten[k * 128:(k + 1) * 128,
                                                    mb * 512:(mb + 1) * 512])
                                    for m in range(4):
                                        nc.tensor.matmul(
                                            psl[m][:, 0:TOK],
                                            wt[:, m * 128:(m + 1) * 128],
                                            hT[k][:], start=(k == 0),
                                            stop=(k == KD - 1))
                                for m in range(4):
                                    kd = mb * 4 + m
                                    raw = ph2w.tile([128, TOK], BF16,
                                                    name="rawqk", tag="rawqk")
                                    nc.scalar.activation(
                                        raw[:], psl[m][:, 0:TOK], AF.Identity,
                                        bias=bias_t[:, kd:kd + 1], scale=scale_)
                                    rope(dstl[kd][:], raw[:])
                                    if send:
                                        nc.sync.dma_start(
                                            snd_k[kd * 128 * TOK:
                                                  (kd + 1) * 128 * TOK]
                                            .rearrange("(p t) -> p t", t=TOK),
                                            dstl[kd][:])

                        # k first: its gather starts while v and q compute
                        proj_fmajor(wk, bkp, 1.0, krT, True)
                        nc.gpsimd.collective_compute(
                            "AllGather", ALU.bypass,
                            replica_groups=[[0, 1, 2, 3], [4, 5, 6, 7]],
                            ins=[snd_k[:]], outs=[gat_k[:]])

                        # v token-major, then its gather
                        for nd in range(NDC):
                            psl = [ps2.tile([128, DCH], F32, name=f"mm{t}",
                                            tag=f"mm{t}") for t in range(NT)]
                            for k in range(KD):
                                wt = wsp.tile([128, 512], BF16, name="wt",
                                              tag="w")
                                nc.sync.dma_start(
                                    wt[:], wv[k * 128:(k + 1) * 128,
                                              nd * 512:(nd + 1) * 512])
                                for t in range(NT):
                                    nc.tensor.matmul(
                                        psl[t][:],
                                        hT[k][:, t * 128:(t + 1) * 128], wt[:],
                                        start=(k == 0), stop=(k == KD - 1))
                            for t in range(NT):
                                nc.scalar.copy(
                                    vtok[t][:, nd * 512:(nd + 1) * 512],
                                    psl[t][:])
                        for t in range(NT):
                            nc.sync.dma_start(
                                snd_v[:].rearrange("(a d) -> a d", d=D)
                                [t * 128:(t + 1) * 128, :], vtok[t][:])
                        nc.gpsimd.collective_compute(
                            "AllGather", ALU.bypass,
                            replica_groups=[[0, 1, 2, 3], [4, 5, 6, 7]],
                            ins=[snd_v[:]], outs=[gat_v[:]])

                        # q last: overlaps the gathers
                        proj_fmajor(wq, bqc, qscale, qrT, False)

                    # ===== phase 3: attention
                    # part B (the core's own causal diagonal) runs for ALL
                    # heads first -- it needs no gathered data, so it
                    # overlaps the k/v AllGathers; per-head partial
                    # (sum p*v, sum p) pairs are combined with part A after
                    # the gathers land.
                    with tc.tile_pool(name="ph3b", bufs=1) as ph3b, \
                         tc.tile_pool(name="ph3w", bufs=3) as ph3w, \
                         tc.tile_pool(name="ps3", bufs=1, space="PSUM") as ps3:
                        ctxB = [ph3b.tile([128, TOK], BF16, name=f"ctxB_{h}")
                                for h in range(H)]
                        lB_d = dramp.tile([H * TOK], F32, name="lB_d")

                        def qk_av(h, avps, lps, lhs_k, lhs_v, bias_ap,
                                  first, last, diag):
                            sps = ps3.tile([128, TOK], F32, name="sps",
                                           tag="sps", bufs=2)
                            nc.tensor.matmul(sps[:], lhs_k, qrT[h][:],
                                             start=True, stop=True)
                            if diag is not None:
                                nc.vector.tensor_add(
                                    sps[:, diag * 128:(diag + 1) * 128],
                                    sps[:, diag * 128:(diag + 1) * 128],
                                    tri[:])
                            p = ph3w.tile([128, TOK], BF16, name="p", tag="p")
                            nc.scalar.activation(p[:], sps[:], AF.Exp,
                                                 bias=bias_ap)
                            if diag is not None and diag > 0:
                                nc.vector.memset(p[:, 0:diag * 128], 0.0)
                            nc.tensor.matmul(lps[:], ones_col[:], p[:],
                                             start=first, stop=last)
                            nc.tensor.matmul(avps[:], lhs_v, p[:],
                                             start=first, stop=last)

                        for h in range(H):
                            avpsB = ps3.tile([128, TOK], F32, name="avpsB",
                                             tag="avpsB", bufs=1)
                            lpsB = ps3.tile([1, TOK], F32, name="lpsB",
                                            tag="lpsB", bufs=1)
                            for kbl in range(NT):
                                qk_av(h, avpsB, lpsB,
                                      krT[h][:, kbl * 128:(kbl + 1) * 128],
                                      vtok[kbl][:, h * 128:(h + 1) * 128],
                                      kbo_bias[:, kbl:kbl + 1],
                                      kbl == 0, kbl == NT - 1, kbl)
                            nc.scalar.copy(ctxB[h][:], avpsB[:])
                            ltmp = ph3w.tile([1, TOK], F32, name="ltmp",
                                             tag="ltmp", bufs=2)
                            nc.scalar.copy(ltmp[:], lpsB[:])
                            nc.sync.dma_start(
                                lB_d[h * TOK:(h + 1) * TOK]
                                .rearrange("(o t) -> o t", o=1), ltmp[:])

                        NA = NKB - NT
                        for h in range(H):
                            avps = ps3.tile([128, TOK], F32, name="avps",
                                            tag="avps", bufs=2)
                            lps = ps3.tile([1, TOK], F32, name="lps",
                                           tag="lps", bufs=1)
                            for j in range(GPC - 1):
                                ktb = ph3w.tile([128, TOK], BF16, name="ktb",
                                                tag="ktb")
                                nc.sync.dma_start(
                                    ktb[:],
                                    gat_k[j, :]
                                    .rearrange("(d t) -> d t", t=TOK)
                                    [h * 128:(h + 1) * 128, :])
                                vtb = ph3w.tile([128, TOK], BF16, name="vtb",
                                                tag="vtb")
                                nc.sync.dma_start(
                                    vtb[:].rearrange("p (a d) -> p a d", a=NT),
                                    gat_v[j, :]
                                    .rearrange("(a p d) -> p a d", p=128, d=D)
                                    [:, :, h * 128:(h + 1) * 128])
                                for kbl in range(NT):
                                    kb = j * NT + kbl
                                    qk_av(h, avps, lps,
                                          ktb[:, kbl * 128:(kbl + 1) * 128],
                                          vtb[:, kbl * 128:(kbl + 1) * 128],
                                          kb_bias[:, kb:kb + 1],
                                          kb == 0, kb == NA - 1, None)

                            lbh = ph3w.tile([1, TOK], F32, name="lbh",
                                            tag="lbh", bufs=2)
                            nc.sync.dma_start(
                                lbh[:], lB_d[h * TOK:(h + 1) * TOK]
                                .rearrange("(o t) -> o t", o=1))
                            lsb = ph3w.tile([1, TOK], F32, name="lsb",
                                            tag="lsb")
                            nc.vector.tensor_add(lsb[:], lps[:], lbh[:])
                            lrep = ps3.tile([128, TOK], F32, name="lrep",
                                            tag="lrep", bufs=1)
                            nc.tensor.matmul(lrep[:], ones_row[:], lsb[:],
                                             start=True, stop=True)
                            linv = ph3w.tile([128, TOK], F32, name="linv",
                                             tag="linv", bufs=2)
                            nc.vector.reciprocal(linv[:], lrep[:])
                            avf = ph3w.tile([128, TOK], F32, name="avf",
                                            tag="avf", bufs=2)
                            nc.vector.tensor_add(avf[:], avps[:], ctxB[h][:])
                            nc.vector.tensor_mul(ctxT[h][:], avf[:], linv[:])

            # ===== phase 4: Wo + residual -> x2
            with tc.tile_pool(name="ph4w", bufs=3) as ph4w, \
                 tc.tile_pool(name="ps4", bufs=2, space="PSUM") as ps4:
                for nd in range(NDC):
                    psl = [ps4.tile([128, DCH], F32, name=f"mm{t}",
                                    tag=f"mm{t}") for t in range(NT)]
                    for k in range(KD):
                        wt = wsp.tile([128, 512], BF16, name="wt", tag="w")
                        nc.sync.dma_start(
                            wt[:], wo[k * 128:(k + 1) * 128,
                                      nd * 512:(nd + 1) * 512])
                        for t in range(NT):
                            nc.tensor.matmul(
                                psl[t][:], ctxT[k][:, t * 128:(t + 1) * 128],
                                wt[:], start=(k == 0), stop=(k == KD - 1))
                    for t in range(NT):
                        xf = ph4w.tile([128, DCH], F32, name="xf", tag="xf")
                        nc.sync.dma_start(
                            xf[:], x_in[t * 128:(t + 1) * 128,
                                        nd * DCH:(nd + 1) * DCH])
                        tt1 = ph4w.tile([128, DCH], F32, name="tt1", tag="tt1")
                        nc.vector.tensor_add(tt1[:], psl[t][:], xf[:])
                        nc.vector.tensor_add(
                            x2_t[t][:, nd * DCH:(nd + 1) * DCH], tt1[:],
                            bo_rep[:, nd * DCH:(nd + 1) * DCH])
                for t in range(NT):
                    nc.vector.tensor_reduce(sums_x2[t][:], x2_t[t][:],
                                            axis=AX.X, op=ALU.add)

        # ===== phases 5-7: LN, FFN, output
        with tc.tile_pool(name="ffnp", bufs=1) as ffnp:
            h2T = [ffnp.tile([128, TOK], BF16, name=f"h2T_{k}")
                   for k in range(KD)]
            uT = [ffnp.tile([128, TOK], BF16, name=f"uT_{k}")
                  for k in range(KF)]
            sT = [ffnp.tile([128, TOK], BF16, name=f"sT_{k}")
                  for k in range(KF)]

            with tc.tile_pool(name="ph5w", bufs=2) as ph5w, \
                 tc.tile_pool(name="ps5", bufs=4, space="PSUM") as ps5:
                for t in range(NT):
                    nmu = ph5w.tile([128, 1], F32, name="nmu", tag="nmu")
                    nc.vector.tensor_scalar(nmu[:], sums_x2[t][:], -1.0 / D,
                                            None, op0=ALU.mult)
                    ss = ph5w.tile([128, NDC], F32, name="ss5", tag="ss5")
                    sq = ph5w.tile([128, DCH], F32, name="sq5", tag="sq5")
                    for c in range(NDC):
                        nc.scalar.activation(
                            sq[:], x2_t[t][:, c * DCH:(c + 1) * DCH],
                            AF.Square, bias=nmu[:], accum_out=ss[:, c:c + 1])
                    var = ph5w.tile([128, 1], F32, name="var", tag="var")
                    nc.vector.tensor_reduce(var[:], ss[:], axis=AX.X,
                                            op=ALU.add)
                    nc.vector.tensor_scalar(var[:], var[:], 1.0 / D, LN_EPS,
                                            op0=ALU.mult, op1=ALU.add)
                    nc.scalar.sqrt(var[:], var[:])
                    rs = ph5w.tile([128, 1], F32, name="rs5", tag="rs5")
                    nc.vector.reciprocal(rs[:], var[:])
                    nrs = ph5w.tile([128, 1], F32, name="nrs", tag="nrs")
                    nc.vector.tensor_mul(nrs[:], nmu[:], rs[:])
                    h2 = ph5w.tile([128, D], BF16, name="h2", tag="h2")
                    nc.scalar.activation(h2[:], x2_t[t][:], AF.Identity,
                                         bias=nrs[:], scale=rs[:])
                    for k in range(KD):
                        tp = ps5.tile([128, 128], BF16, name="tp5", tag="tp5")
                        nc.tensor.transpose(tp[:], h2[:, k * 128:(k + 1) * 128],
                                            ident[:])
                        nc.scalar.copy(h2T[k][:, t * 128:(t + 1) * 128], tp[:])

            with tc.tile_pool(name="ph6w", bufs=2) as ph6w, \
                 tc.tile_pool(name="ps6", bufs=2, space="PSUM") as ps6:
                for mb in range(KF // 4):
                    psl = [ps6.tile([128, TOK], F32, name=f"mm{m}",
                                    tag=f"mm{m}") for m in range(4)]
                    for k in range(KD):
                        wt = wsp.tile([128, 512], BF16, name="wt", tag="w")
                        nc.sync.dma_start(
                            wt[:], w1[k * 128:(k + 1) * 128,
                                      mb * 512:(mb + 1) * 512])
                        for m in range(4):
                            nc.tensor.matmul(
                                psl[m][:], wt[:, m * 128:(m + 1) * 128],
                                h2T[k][:], start=(k == 0), stop=(k == KD - 1))
                    for m in range(4):
                        kf = mb * 4 + m
                        nc.scalar.activation(uT[kf][:], psl[m][:], AF.Identity,
                                             bias=b1t[:, kf:kf + 1])

                for mb in range(KF // 4):
                    g1l = [ph6w.tile([128, TOK], BF16, name=f"g1_{m}",
                                     tag=f"g1_{m}") for m in range(4)]
                    psl = [ps6.tile([128, TOK], F32, name=f"mm{m}",
                                    tag=f"mm{m}") for m in range(4)]
                    for k in range(KF):
                        wt = wsp.tile([128, 512], BF16, name="wt", tag="w")
                        nc.sync.dma_start(
                            wt[:], wg1[k * 128:(k + 1) * 128,
                                       mb * 512:(mb + 1) * 512])
                        for m in range(4):
                            nc.tensor.matmul(
                                psl[m][:], wt[:, m * 128:(m + 1) * 128],
                                uT[k][:], start=(k == 0), stop=(k == KF - 1))
                    for m in range(4):
                        kf = mb * 4 + m
                        sg = ph6w.tile([128, TOK], BF16, name="sg", tag="sg")
                        nc.scalar.activation(sg[:], psl[m][:], AF.Sigmoid,
                                             bias=bg1t[:, kf:kf + 1])
                        g1b = ph6w.tile([128, TOK], BF16, name="g1b",
                                        tag="g1b")
                        nc.scalar.activation(g1b[:], psl[m][:], AF.Identity,
                                             bias=bg1t[:, kf:kf + 1])
                        nc.vector.tensor_mul(g1l[m][:], sg[:], g1b[:])
                    psl2 = [ps6.tile([128, TOK], F32, name=f"mm{m}",
                                     tag=f"mm{m}") for m in range(4)]
                    for k in range(KF):
                        wt = wsp.tile([128, 512], BF16, name="wt", tag="w")
                        nc.sync.dma_start(
                            wt[:], wg2[k * 128:(k + 1) * 128,
                                       mb * 512:(mb + 1) * 512])
                        for m in range(4):
                            nc.tensor.matmul(
                                psl2[m][:], wt[:, m * 128:(m + 1) * 128],
                                uT[k][:], start=(k == 0), stop=(k == KF - 1))
                    for m in range(4):
                        kf = mb * 4 + m
                        nc.vector.scalar_tensor_tensor(
                            sT[kf][:], psl2[m][:], bg2t[:, kf:kf + 1],
                            g1l[m][:], op0=ALU.add, op1=ALU.mult)

            with tc.tile_pool(name="ph7w", bufs=3) as ph7w, \
                 tc.tile_pool(name="ps7", bufs=2, space="PSUM") as ps7:
                for nd in range(NDC):
                    psl = [ps7.tile([128, DCH], F32, name=f"mm{t}",
                                    tag=f"mm{t}") for t in range(NT)]
                    for k in range(KF):
                        wt = wsp.tile([128, 512], BF16, name="wt", tag="w")
                        nc.sync.dma_start(
                            wt[:], w2[k * 128:(k + 1) * 128,
                                      nd * 512:(nd + 1) * 512])
                        for t in range(NT):
                            nc.tensor.matmul(
                                psl[t][:], sT[k][:, t * 128:(t + 1) * 128],
                                wt[:], start=(k == 0), stop=(k == KF - 1))
                    for t in range(NT):
                        tt1 = ph7w.tile([128, DCH], F32, name="o1", tag="o1")
                        nc.vector.tensor_add(
                            tt1[:], psl[t][:],
                            x2_t[t][:, nd * DCH:(nd + 1) * DCH])
                        yf = ph7w.tile([128, DCH], F32, name="yf", tag="yf")
                        nc.vector.tensor_add(
                            yf[:], tt1[:], b2_rep[:, nd * DCH:(nd + 1) * DCH])
                        nc.sync.dma_start(
                            out_d[t * 128:(t + 1) * 128,
                                  nd * DCH:(nd + 1) * DCH], yf[:])
    n = split_excess_waits(nc)
    return nc


# ---------------------------------------------------------------- host side


def host_prepare(inputs, cfg):
    B, T, D, H, DFF = cfg["B"], cfg["T"], cfg["D"], cfg["H"], cfg["DFF"]
    dv = derived(cfg)
    HD, TOK = dv["HD"], dv["TOK"]
    f32 = np.float32
    bf = ml_dtypes.bfloat16

    x = np.asarray(inputs["x"], f32)
    g_rms = np.asarray(inputs["g_rms"], f32)
    g_ln = np.asarray(inputs["g_ln"], f32)
    b_ln = np.asarray(inputs["b_ln"], f32)
    pad = np.asarray(inputs["pad_mask"])

    perm = np.concatenate(
        [h * HD + np.concatenate([np.arange(0, HD, 2), np.arange(1, HD, 2)])
         for h in range(H)])
    wq = (g_rms[:, None] * np.asarray(inputs["Wq"], f32))[:, perm].astype(bf)
    wk = (g_rms[:, None] * np.asarray(inputs["Wk"], f32))[:, perm].astype(bf)
    wv = (g_rms[:, None] * np.asarray(inputs["Wv"], f32)).astype(bf)
    wo = np.asarray(inputs["Wo"], f32).astype(bf)
    w1 = (g_ln[:, None] * np.asarray(inputs["W1"], f32)).astype(bf)
    wg1 = np.asarray(inputs["Wg1"], f32).astype(bf)
    wg2 = np.asarray(inputs["Wg2"], f32).astype(bf)
    w2 = np.asarray(inputs["W2"], f32).astype(bf)

    qscale = 1.0 / math.sqrt(HD)
    bqc = (np.asarray(inputs["bq"], f32)[perm] * qscale).astype(f32)
    bkp = np.asarray(inputs["bk"], f32)[perm].astype(f32)
    b1p = (np.asarray(inputs["b1"], f32)
           + b_ln @ np.asarray(inputs["W1"], f32)).astype(f32)
    bg1 = np.asarray(inputs["bg1"], f32)
    bg2 = np.asarray(inputs["bg2"], f32)
    bo_rep = np.broadcast_to(np.asarray(inputs["bo"], f32), (128, D)).copy()
    b2_rep = np.broadcast_to(np.asarray(inputs["b2"], f32), (128, D)).copy()

    inv_freq = 1.0 / (10000.0 ** (np.arange(0, HD, 2, dtype=f32) / HD))
    ang = np.arange(T, dtype=f32)[:, None] * inv_freq[None, :]
    cosA, sinA = np.cos(ang).astype(f32), np.sin(ang).astype(f32)

    tri = np.where(np.arange(128)[:, None] <= np.arange(128)[None, :],
                   np.float32(0.0), np.float32(NEG))

    in_maps = []
    for i in range(CORES):
        g, p = i // GPC, i % GPC
        t0 = p * TOK
        kb = np.where(pad[g] == 0, np.float32(NEG), np.float32(0.0))
        kb[t0:] = NEG
        kbo = np.where(pad[g, t0:t0 + TOK] == 0, np.float32(NEG),
                       np.float32(0.0))
        in_maps.append(dict(
            x=np.ascontiguousarray(x[g, t0:t0 + TOK]),
            wq=wq, wk=wk, wv=wv, wo=wo, w1=w1, wg1=wg1, wg2=wg2, w2=w2,
            bqc=bqc, bkp=bkp, b1p=b1p, bg1=bg1, bg2=bg2,
            bo_rep=bo_rep, b2_rep=b2_rep,
            cosT=np.ascontiguousarray(
                np.tile(cosA[t0:t0 + TOK].T, (2, 1))),
            sinT=np.ascontiguousarray(
                np.tile(sinA[t0:t0 + TOK].T, (2, 1))),
            keybias=kb, keybias_own=kbo, triT=tri,
        ))
    return in_maps


def host_assemble(results, cfg):
    B, T, D = cfg["B"], cfg["T"], cfg["D"]
    TOK = derived(cfg)["TOK"]
    out = np.empty((B, T, D), np.float32)
    for i in range(CORES):
        g, p = i // GPC, i % GPC
        out[g, p * TOK:(p + 1) * TOK] = results[i]["out"]
    return out


# ---------------------------------------------------------------- numpy ref


def numpy_reference(inputs, cfg):
    B, T, D, H, DFF = cfg["B"], cfg["T"], cfg["D"], cfg["H"], cfg["DFF"]
    HD = D // H
    f = np.float32
    x = np.asarray(inputs["x"], f)
    RMS_EPS = float(np.finfo(np.float32).eps)

    h = x * (1.0 / np.sqrt((x * x).mean(-1, keepdims=True) + RMS_EPS))
    h = h * inputs["g_rms"]
    q = (h @ inputs["Wq"] + inputs["bq"]).reshape(B, T, H, HD).transpose(0, 2, 1, 3)
    k = (h @ inputs["Wk"] + inputs["bk"]).reshape(B, T, H, HD).transpose(0, 2, 1, 3)
    v = (h @ inputs["Wv"]).reshape(B, T, H, HD).transpose(0, 2, 1, 3)

    inv_freq = 1.0 / (10000.0 ** (np.arange(0, HD, 2, dtype=f) / HD))
    ang = np.arange(T, dtype=f)[:, None] * inv_freq[None, :]
    cos, sin = np.cos(ang), np.sin(ang)

    def rope(z):
        z1, z2 = z[..., ::2], z[..., 1::2]
        out = np.stack([z1 * cos - z2 * sin, z1 * sin + z2 * cos], -1)
        return out.reshape(z.shape)

    q, k = rope(q), rope(k)
    scores = np.einsum("bhqd,bhkd->bhqk", q, k) / np.sqrt(np.float32(HD))
    causal = np.tril(np.ones((T, T), bool))
    mask = (np.asarray(inputs["pad_mask"])[:, None, :].astype(bool)
            & causal)[:, None]
    scores = np.where(mask, scores, -np.inf)
    m = scores.max(-1, keepdims=True)
    e = np.exp(scores - m)
    attn = e / e.sum(-1, keepdims=True)
    o = np.einsum("bhqk,bhkd->bhqd", attn, v)
    o = o.transpose(0, 2, 1, 3).reshape(B, T, D)
    x = x + o @ inputs["Wo"] + inputs["bo"]

    mu = x.mean(-1, keepdims=True)
    var = ((x - mu) ** 2).mean(-1, keepdims=True)
    h2 = (x - mu) / np.sqrt(var + 1e-5) * inputs["g_ln"] + inputs["b_ln"]
    u = h2 @ inputs["W1"] + inputs["b1"]
    g1 = u @ inputs["Wg1"] + inputs["bg1"]
    s = (g1 / (1 + np.exp(-g1))) * (u @ inputs["Wg2"] + inputs["bg2"])
    return x + s @ inputs["W2"] + inputs["b2"]


def make_small_inputs(cfg, seed=0):
    B, T, D, H, DFF = cfg["B"], cfg["T"], cfg["D"], cfg["H"], cfg["DFF"]
    rng = np.random.default_rng(seed)
    f = np.float32

    def w(shape, fan):
        return ((rng.random(shape, dtype=f) * 2 - 1) / np.sqrt(fan)).astype(f)

    lengths = rng.integers(T // 2, T + 1, size=(B,))
    pad = (np.arange(T)[None, :] < lengths[:, None]).astype(np.int32)
    return dict(
        x=rng.standard_normal((B, T, D), dtype=f),
        Wq=w((D, D), D), bq=rng.standard_normal(D, dtype=f) * 0.02,
        Wk=w((D, D), D), bk=rng.standard_normal(D, dtype=f) * 0.02,
        Wv=w((D, D), D),
        Wo=w((D, D), D), bo=rng.standard_normal(D, dtype=f) * 0.02,
        W1=w((D, DFF), D), b1=rng.standard_normal(DFF, dtype=f) * 0.02,
        Wg1=w((DFF, DFF), DFF), bg1=rng.standard_normal(DFF, dtype=f) * 0.02,
        Wg2=w((DFF, DFF), DFF), bg2=rng.standard_normal(DFF, dtype=f) * 0.02,
        W2=w((DFF, D), DFF), b2=rng.standard_normal(D, dtype=f) * 0.02,
        g_rms=(1 + 0.1 * rng.standard_normal(D)).astype(f),
        g_ln=(1 + 0.1 * rng.standard_normal(D)).astype(f),
        b_ln=(0.05 * rng.standard_normal(D)).astype(f),
        pad_mask=pad,
    )


# ===================== tile scheduler patch =====================


import concourse.tile as tile


def _split_drain_and_barrier(self, tick_clock, wait_clock):
    from concourse.vector_clock import ScopedClock

    drain_inst = self.nc.sync.drain()
    wait_clock.add_sem_waits(
        drain_inst.ins, ScopedClock({None: tick_clock.global_clock})
    )
    si = drain_inst.ins.sync_info
    waits = list(si.on_wait) if si and si.on_wait else []
    if len(waits) > 1:
        si.on_wait.clear()
        si.on_wait.extend(waits[:1])
        for i in range(1, len(waits), 1):
            extra = self.nc.sync.drain()
            esi = extra.ins.sync_info
            if esi is None:
                import concourse.mybir as mybir

                extra.ins.sync_info = mybir.SyncInfo(
                    on_wait=waits[i : i + 1], on_update=[]
                )
            else:
                esi.on_wait.extend(waits[i : i + 1])

    self.nc.all_engine_barrier()
    assert self.sems is not None
    popped = self.nc._tile_sem_poison_stack.pop()
    assert popped is self._sem_poison
    self.nc.clear_and_free_semaphores(list(self.sems.allocated().values()))
    self.nc.all_engine_barrier()


def split_excess_waits(nc, default_limit=1, ctrl_limit=1, dma_limit=1):
    """Walrus in this container rejects instructions whose sync_info
    carries more wait commands than the ISA encoding has slots for.
    Move excess waits onto same-engine no-op carriers inserted right
    before the offending instruction (engine queues are in-order, so the
    carrier's waits are observed before the instruction issues)."""
    import concourse.mybir as mybir

    CTRL = ("InstDrain", "InstNoOp", "InstEventSemaphore")
    DMA = ("InstDMACopy", "InstTriggeredCopy", "InstDMATranspose")
    nsplit = 0
    for bb_name, bbw in list(nc.bb_map.items()):
        bb = bbw.bb if hasattr(bbw, "bb") else bbw
        insts = bb.instructions
        i = 0
        while i < len(insts):
            inst = insts[i]
            tname = type(inst).__name__
            limit = (ctrl_limit if tname in CTRL
                     else dma_limit if tname in DMA else default_limit)
            si = inst.sync_info
            waits = list(si.on_wait) if si and si.on_wait else []
            if len(waits) > limit:
                keep, extra = waits[:limit], waits[limit:]
                si.on_wait.clear()
                si.on_wait.extend(keep)
                ncar = 0
                for j in range(0, len(extra), ctrl_limit):
                    chunk = extra[j:j + ctrl_limit]
                    car = nc.engines[inst.engine].nop(nofuse=True).ins
                    # nop() appended to the current bb; move it here
                    for other in nc.bb_map.values():
                        obb = other.bb if hasattr(other, "bb") else other
                        if obb.instructions and obb.instructions[-1] is car:
                            obb.instructions.pop()
                            break
                    car.sync_info = mybir.SyncInfo(on_wait=chunk, on_update=[])
                    insts.insert(i, car)
                    ncar += 1
                i += ncar
                nsplit += 1
            i += 1
    return nsplit


def _apply_tile_patch():
    tile.TileContext._drain_and_barrier = _split_drain_and_barrier


# ================================================================ runner

_tile_patch_applied = False
_build_cache = {}
LAST_EXEC_NS = None


def _get_nc():
    global _tile_patch_applied
    if not _tile_patch_applied:
        _apply_tile_patch()
        _tile_patch_applied = True
    if "nc" not in _build_cache:
        nc = bass.Bass()
        build(nc, full_cfg())
        _build_cache["nc"] = nc
    return _build_cache["nc"]


def kernel(_profile=False, **inputs):
    """Full-input decoder block on 8 TRN2 NeuronCores.

    inputs: the arrays from reference.setup_inputs() (numpy or jax).
    Returns the full [B, T, D] float32 output.
    """
    global LAST_EXEC_NS
    from concourse.bass_utils import run_bass_kernel_spmd

    cfg = full_cfg()
    nc = _get_nc()
    in_maps = host_prepare({k: np.asarray(v) for k, v in inputs.items()}, cfg)
    res = run_bass_kernel_spmd(nc, in_maps, list(range(CORES)),
                               trace=bool(_profile))
    LAST_EXEC_NS = getattr(res, "exec_time_ns", None)
    return host_assemble(res.results, cfg)



# revision 5
# speedup vs baseline: 1.5249x; 1.5249x over previous
"""nn_DecoderBlock Trainium2 kernel — 8 NeuronCores, token-sharded.

Self-contained: builds a Bass/Tile SPMD program (one program, all 8
cores; per-core differences are input data), runs it via
run_bass_kernel_spmd, reassembles the full output on the host.

All 8 linear layers run in fp8(e4m3) with DoubleRow matmuls (K=256 per
instruction); attention QK/softmax/AV stays bf16/f32.
"""


import math
from contextlib import ExitStack

import numpy as np
import ml_dtypes

import concourse.bass as bass
import concourse.mybir as mybir
from concourse.tile import TileContext
from concourse.masks import make_identity

F32 = mybir.dt.float32
BF16 = mybir.dt.bfloat16
F8 = mybir.dt.float8e4
NP_F8 = ml_dtypes.float8_e4m3
AF = mybir.ActivationFunctionType
ALU = mybir.AluOpType
AX = mybir.AxisListType
DR = mybir.MatmulPerfMode.DoubleRow

NEG = -1.0e9
CORES = 8
GPC = 4

# fp8 scales (powers of two; folded out at PSUM evacuation)
SH = 8.0     # rms-normed h
SH2 = 8.0    # layernormed h2
SU = 8.0     # ffn mid u
SS = 8.0     # swiglu out s
SCTX = 16.0  # attention context
SWD = 32.0   # weights with fan-in D
SWF = 64.0   # weights with fan-in DFF


def full_cfg():
    return dict(B=2, T=2048, D=2048, H=16, DFF=4096)


def small_cfg():
    return dict(B=2, T=512, D=512, H=4, DFF=1024)


def derived(cfg):
    B, T, D, H, DFF = cfg["B"], cfg["T"], cfg["D"], cfg["H"], cfg["DFF"]
    HD = D // H
    assert HD == 128
    TOK = B * T // CORES
    assert T // GPC == TOK and TOK % 128 == 0
    return dict(HD=HD, TOK=TOK, NT=TOK // 128, KD=D // 128, KF=DFF // 128,
                NKB=T // 128, KGD=D // 256, KGF=DFF // 256)


def build(nc: bass.Bass, cfg):
    B, T, D, H, DFF = cfg["B"], cfg["T"], cfg["D"], cfg["H"], cfg["DFF"]
    dv = derived(cfg)
    TOK, NT, KD, KF, NKB = dv["TOK"], dv["NT"], dv["KD"], dv["KF"], dv["NKB"]
    KGD, KGF = dv["KGD"], dv["KGF"]
    DCH = min(512, D)
    NDC = D // DCH
    NFC = DFF // DCH
    RMS_EPS = float(np.finfo(np.float32).eps)
    LN_EPS = 1e-5
    DT = D * TOK

    x_in = nc.declare_dram_parameter("x", [TOK, D], F32, isOutput=False)
    # fp8 weights, packed as [(KG*NCH)*128, 2*512] row-pair chunks
    wq = nc.declare_dram_parameter("wq", [KGD * NDC * 128, 2 * DCH], F8, isOutput=False)
    wk = nc.declare_dram_parameter("wk", [KGD * NDC * 128, 2 * DCH], F8, isOutput=False)
    wv = nc.declare_dram_parameter("wv", [KGD * NDC * 128, 2 * DCH], F8, isOutput=False)
    wo = nc.declare_dram_parameter("wo", [KGD * NDC * 128, 2 * DCH], F8, isOutput=False)
    w1 = nc.declare_dram_parameter("w1", [KGD * NFC * 128, 2 * DCH], F8, isOutput=False)
    wg1 = nc.declare_dram_parameter("wg1", [KGF * NFC * 128, 2 * DCH], F8, isOutput=False)
    wg2 = nc.declare_dram_parameter("wg2", [KGF * NFC * 128, 2 * DCH], F8, isOutput=False)
    w2 = nc.declare_dram_parameter("w2", [KGF * NDC * 128, 2 * DCH], F8, isOutput=False)
    bqc_d = nc.declare_dram_parameter("bqc", [D], F32, isOutput=False)
    bkp_d = nc.declare_dram_parameter("bkp", [D], F32, isOutput=False)
    b1_d = nc.declare_dram_parameter("b1p8", [DFF], F32, isOutput=False)
    bg1_d = nc.declare_dram_parameter("bg1", [DFF], F32, isOutput=False)
    bg2_d = nc.declare_dram_parameter("bg28", [DFF], F32, isOutput=False)
    bo_rep_d = nc.declare_dram_parameter("bo_rep", [128, D], F32, isOutput=False)
    b2_rep_d = nc.declare_dram_parameter("b2_rep", [128, D], F32, isOutput=False)
    cos_d = nc.declare_dram_parameter("cosT", [128, TOK], F32, isOutput=False)
    sin_d = nc.declare_dram_parameter("sinT", [128, TOK], F32, isOutput=False)
    keybias_d = nc.declare_dram_parameter("keybias", [T], F32, isOutput=False)
    kbown_d = nc.declare_dram_parameter("keybias_own", [TOK], F32, isOutput=False)
    tri_d = nc.declare_dram_parameter("triT", [128, 128], F32, isOutput=False)
    out_d = nc.declare_dram_parameter("out", [TOK, D], F32, isOutput=True)

    with TileContext(nc) as tc, ExitStack() as top:
        constp = top.enter_context(tc.tile_pool(name="constp", bufs=1))
        dramp = top.enter_context(tc.tile_pool(name="dramp", bufs=1, space="DRAM"))
        wsp = top.enter_context(tc.tile_pool(name="wsp", bufs=16))
        x2p = top.enter_context(tc.tile_pool(name="x2p", bufs=1))

        # ---- constants
        ident = constp.tile([128, 128], BF16, name="ident")
        make_identity(nc, ident[:])
        ones_col = constp.tile([128, 1], BF16, name="ones_col")
        nc.vector.memset(ones_col[:], 1.0)
        ones_row = constp.tile([1, 128], F32, name="ones_row")
        nc.vector.memset(ones_row[:], 1.0)
        tri = constp.tile([128, 128], F32, name="tri")
        nc.sync.dma_start(tri[:], tri_d[:])
        cosT = constp.tile([128, TOK], F32, name="cosT")
        sinT = constp.tile([128, TOK], F32, name="sinT")
        nc.sync.dma_start(cosT[:], cos_d[:])
        nc.sync.dma_start(sinT[:], sin_d[:])
        kb_bias = constp.tile([128, NKB], F32, name="kb_bias")
        nc.sync.dma_start(kb_bias[:], keybias_d[:].rearrange("(n p) -> p n", p=128))
        kbo_bias = constp.tile([128, NT], F32, name="kbo_bias")
        nc.sync.dma_start(kbo_bias[:], kbown_d[:].rearrange("(n p) -> p n", p=128))
        bqc = constp.tile([128, KD], F32, name="bqc")
        nc.sync.dma_start(bqc[:], bqc_d[:].rearrange("(n p) -> p n", p=128))
        bkp = constp.tile([128, KD], F32, name="bkp")
        nc.sync.dma_start(bkp[:], bkp_d[:].rearrange("(n p) -> p n", p=128))
        b1t = constp.tile([128, KF], F32, name="b1t")
        nc.sync.dma_start(b1t[:], b1_d[:].rearrange("(n p) -> p n", p=128))
        bg1t = constp.tile([128, KF], F32, name="bg1t")
        nc.sync.dma_start(bg1t[:], bg1_d[:].rearrange("(n p) -> p n", p=128))
        bg2t = constp.tile([128, KF], F32, name="bg2t")
        nc.sync.dma_start(bg2t[:], bg2_d[:].rearrange("(n p) -> p n", p=128))
        bo_rep = constp.tile([128, D], F32, name="bo_rep")
        nc.sync.dma_start(bo_rep[:], bo_rep_d[:])
        b2_rep = constp.tile([128, D], F32, name="b2_rep")
        nc.sync.dma_start(b2_rep[:], b2_rep_d[:])

        snd_k = dramp.tile([DT], BF16, name="snd_k")
        snd_v = dramp.tile([DT], BF16, name="snd_v")
        gat_k = dramp.tile([GPC, DT], BF16, name="gat_k")
        gat_v = dramp.tile([GPC, DT], BF16, name="gat_v")

        x2_t = [x2p.tile([128, D], F32, name=f"x2_{t}") for t in range(NT)]
        sums_x2 = [x2p.tile([128, 1], F32, name=f"sx2_{t}") for t in range(NT)]

        with tc.tile_pool(name="ctxp", bufs=1) as ctxp:
            ctxT = [ctxp.tile([128, 2, TOK], F8, name=f"ctxT_{g}")
                    for g in range(KGD)]

            with tc.tile_pool(name="hTp", bufs=1) as hTp:
                hT = [hTp.tile([128, 2, TOK], F8, name=f"hT_{g}")
                      for g in range(KGD)]

                # ===== phase 1: RMSNorm + transpose -> hT (fp8, x SH)
                with tc.tile_pool(name="ph1w", bufs=2) as ph1w, \
                     tc.tile_pool(name="ps1", bufs=4, space="PSUM") as ps1:
                    for t in range(NT):
                        xt = ph1w.tile([128, D], F32, name="xt", tag="xt")
                        nc.sync.dma_start(xt[:], x_in[t * 128:(t + 1) * 128, :])
                        ss = ph1w.tile([128, NDC], F32, name="ss", tag="ss")
                        sq = ph1w.tile([128, DCH], F32, name="sq", tag="sq")
                        for c in range(NDC):
                            nc.scalar.activation(
                                sq[:], xt[:, c * DCH:(c + 1) * DCH], AF.Square,
                                accum_out=ss[:, c:c + 1])
                        ssum = ph1w.tile([128, 1], F32, name="ssum", tag="ssum")
                        nc.vector.tensor_reduce(ssum[:], ss[:], axis=AX.X,
                                                op=ALU.add)
                        # sqrt((mean+eps)/SH^2) -> recip = SH * rsqrt(mean+eps)
                        nc.vector.tensor_scalar(
                            ssum[:], ssum[:], 1.0 / (D * SH * SH),
                            RMS_EPS / (SH * SH), op0=ALU.mult, op1=ALU.add)
                        nc.scalar.sqrt(ssum[:], ssum[:])
                        rs = ph1w.tile([128, 1], F32, name="rs", tag="rs")
                        nc.vector.reciprocal(rs[:], ssum[:])
                        hn = ph1w.tile([128, D], BF16, name="hn",
                                       tag="hn", bufs=2)
                        nc.scalar.activation(hn[:], xt[:], AF.Copy, scale=rs[:])
                        for k in range(KD):
                            tp = ps1.tile([128, 128], BF16, name="tp", tag="tp")
                            nc.tensor.transpose(
                                tp[:], hn[:, k * 128:(k + 1) * 128], ident[:])
                            nc.scalar.copy(
                                hT[k // 2][:, k % 2, t * 128:(t + 1) * 128],
                                tp[:])

                with tc.tile_pool(name="qkvp", bufs=1) as qkvp:
                    qrT = [qkvp.tile([128, TOK], BF16, name=f"qrT_{k}")
                           for k in range(KD)]
                    krT = [qkvp.tile([128, TOK], BF16, name=f"krT_{k}")
                           for k in range(KD)]
                    vtok = [qkvp.tile([128, D], BF16, name=f"vtok_{t}")
                            for t in range(NT)]

                    # ===== phase 2: projections + rope + send + gather
                    with tc.tile_pool(name="ph2w", bufs=4) as ph2w, \
                         tc.tile_pool(name="ps2", bufs=2, space="PSUM") as ps2:

                        def rope(dst, src):
                            # walrus: SB+SB tensor_tensor operands must share
                            # base partition -> cos/sin are replicated on both
                            # halves and tmps live at base 0
                            t1 = ph2w.tile([64, TOK], F32, name="rp1", tag="rp1")
                            t2 = ph2w.tile([64, TOK], F32, name="rp2", tag="rp2")
                            t3 = ph2w.tile([64, TOK], F32, name="rp3", tag="rp3")
                            t4 = ph2w.tile([64, TOK], F32, name="rp4", tag="rp4")
                            nc.vector.tensor_mul(t1[:], src[0:64, :], cosT[0:64, :])
                            nc.vector.tensor_mul(t2[:], src[64:128, :], sinT[64:128, :])
                            nc.vector.tensor_sub(dst[0:64, :], t1[:], t2[:])
                            nc.vector.tensor_mul(t3[:], src[0:64, :], sinT[0:64, :])
                            nc.vector.tensor_mul(t4[:], src[64:128, :], cosT[64:128, :])
                            nc.vector.tensor_add(dst[64:128, :], t3[:], t4[:])

                        qscale = 1.0 / math.sqrt(128.0)
                        PSCALE = 1.0 / (SH * SWD)

                        def proj_fmajor(wten, bias_t, scale_, dstl, send):
                            for mb in range(KD // 4):
                                psl = [ps2.tile([128, TOK], F32, name=f"mm{m}",
                                                tag=f"mm{m}") for m in range(4)]
                                for kg in range(KGD):
                                    wt = wsp.tile([128, 2, DCH], F8, name="wt",
                                                  tag="w")
                                    base = (kg * NDC + mb) * 128
                                    nc.sync.dma_start(
                                        wt[:], wten[base:base + 128, :]
                                        .rearrange("p (a b) -> p a b", a=2))
                                    for m in range(4):
                                        nc.tensor.matmul(
                                            psl[m][:],
                                            wt[:, :, m * 128:(m + 1) * 128],
                                            hT[kg][:], start=(kg == 0),
                                            stop=(kg == KGD - 1),
                                            perf_mode=DR)
                                for m in range(4):
                                    kd = mb * 4 + m
                                    raw = ph2w.tile([128, TOK], BF16,
                                                    name="rawqk", tag="rawqk")
                                    nc.scalar.activation(
                                        raw[:], psl[m][:], AF.Identity,
                                        bias=bias_t[:, kd:kd + 1],
                                        scale=scale_ * PSCALE)
                                    rope(dstl[kd][:], raw[:])
                                    if send:
                                        nc.sync.dma_start(
                                            snd_k[kd * 128 * TOK:
                                                  (kd + 1) * 128 * TOK]
                                            .rearrange("(p t) -> p t", t=TOK),
                                            dstl[kd][:])

                        # k first: its gather starts while v and q compute
                        proj_fmajor(wk, bkp, 1.0, krT, True)
                        nc.gpsimd.collective_compute(
                            "AllGather", ALU.bypass,
                            replica_groups=[[0, 1, 2, 3], [4, 5, 6, 7]],
                            ins=[snd_k[:]], outs=[gat_k[:]])

                        # v token-major, then its gather
                        for nd in range(NDC):
                            psl = [ps2.tile([128, DCH], F32, name=f"mm{t}",
                                            tag=f"mm{t}") for t in range(NT)]
                            for kg in range(KGD):
                                wt = wsp.tile([128, 2, DCH], F8, name="wt",
                                              tag="w")
                                base = (kg * NDC + nd) * 128
                                nc.sync.dma_start(
                                    wt[:], wv[base:base + 128, :]
                                    .rearrange("p (a b) -> p a b", a=2))
                                for t in range(NT):
                                    nc.tensor.matmul(
                                        psl[t][:],
                                        hT[kg][:, :, t * 128:(t + 1) * 128],
                                        wt[:],
                                        start=(kg == 0), stop=(kg == KGD - 1),
                                        perf_mode=DR)
                            for t in range(NT):
                                nc.scalar.activation(
                                    vtok[t][:, nd * DCH:(nd + 1) * DCH],
                                    psl[t][:], AF.Copy, scale=PSCALE)
                        for t in range(NT):
                            nc.sync.dma_start(
                                snd_v[:].rearrange("(a d) -> a d", d=D)
                                [t * 128:(t + 1) * 128, :], vtok[t][:])
                        nc.gpsimd.collective_compute(
                            "AllGather", ALU.bypass,
                            replica_groups=[[0, 1, 2, 3], [4, 5, 6, 7]],
                            ins=[snd_v[:]], outs=[gat_v[:]])

                        # q last: overlaps the gathers
                        proj_fmajor(wq, bqc, qscale, qrT, False)

                    # ===== phase 3: attention
                    # part B (the core's own causal diagonal) runs for ALL
                    # heads first -- it needs no gathered data, so it
                    # overlaps the k/v AllGathers; per-head partial
                    # (sum p*v, sum p) pairs are combined with part A after
                    # the gathers land.
                    with tc.tile_pool(name="ph3b", bufs=1) as ph3b, \
                         tc.tile_pool(name="ph3w", bufs=3) as ph3w, \
                         tc.tile_pool(name="ps3", bufs=1, space="PSUM") as ps3:
                        ctxB = [ph3b.tile([128, TOK], BF16, name=f"ctxB_{h}")
                                for h in range(H)]
                        lB_d = dramp.tile([H * TOK], F32, name="lB_d")

                        def qk_av(h, avps, lps, lhs_k, lhs_v, bias_ap,
                                  first, last, diag):
                            sps = ps3.tile([128, TOK], F32, name="sps",
                                           tag="sps", bufs=2)
                            nc.tensor.matmul(sps[:], lhs_k, qrT[h][:],
                                             start=True, stop=True)
                            if diag is not None:
                                nc.vector.tensor_add(
                                    sps[:, diag * 128:(diag + 1) * 128],
                                    sps[:, diag * 128:(diag + 1) * 128],
                                    tri[:])
                            p = ph3w.tile([128, TOK], BF16, name="p", tag="p")
                            nc.scalar.activation(p[:], sps[:], AF.Exp,
                                                 bias=bias_ap)
                            if diag is not None and diag > 0:
                                nc.vector.memset(p[:, 0:diag * 128], 0.0)
                            nc.tensor.matmul(lps[:], ones_col[:], p[:],
                                             start=first, stop=last)
                            nc.tensor.matmul(avps[:], lhs_v, p[:],
                                             start=first, stop=last)

                        for h in range(H):
                            avpsB = ps3.tile([128, TOK], F32, name="avpsB",
                                             tag="avpsB", bufs=1)
                            lpsB = ps3.tile([1, TOK], F32, name="lpsB",
                                            tag="lpsB", bufs=1)
                            for kbl in range(NT):
                                qk_av(h, avpsB, lpsB,
                                      krT[h][:, kbl * 128:(kbl + 1) * 128],
                                      vtok[kbl][:, h * 128:(h + 1) * 128],
                                      kbo_bias[:, kbl:kbl + 1],
                                      kbl == 0, kbl == NT - 1, kbl)
                            nc.scalar.copy(ctxB[h][:], avpsB[:])
                            ltmp = ph3w.tile([1, TOK], F32, name="ltmp",
                                             tag="ltmp", bufs=2)
                            nc.scalar.copy(ltmp[:], lpsB[:])
                            nc.sync.dma_start(
                                lB_d[h * TOK:(h + 1) * TOK]
                                .rearrange("(o t) -> o t", o=1), ltmp[:])

                        NA = NKB - NT
                        for h in range(H):
                            avps = ps3.tile([128, TOK], F32, name="avps",
                                            tag="avps", bufs=2)
                            lps = ps3.tile([1, TOK], F32, name="lps",
                                           tag="lps", bufs=1)
                            for j in range(GPC - 1):
                                ktb = ph3w.tile([128, TOK], BF16, name="ktb",
                                                tag="ktb")
                                nc.sync.dma_start(
                                    ktb[:],
                                    gat_k[j, :]
                                    .rearrange("(d t) -> d t", t=TOK)
                                    [h * 128:(h + 1) * 128, :])
                                vtb = ph3w.tile([128, TOK], BF16, name="vtb",
                                                tag="vtb")
                                nc.sync.dma_start(
                                    vtb[:].rearrange("p (a d) -> p a d", a=NT),
                                    gat_v[j, :]
                                    .rearrange("(a p d) -> p a d", p=128, d=D)
                                    [:, :, h * 128:(h + 1) * 128])
                                for kbl in range(NT):
                                    kb = j * NT + kbl
                                    qk_av(h, avps, lps,
                                          ktb[:, kbl * 128:(kbl + 1) * 128],
                                          vtb[:, kbl * 128:(kbl + 1) * 128],
                                          kb_bias[:, kb:kb + 1],
                                          kb == 0, kb == NA - 1, None)

                            lbh = ph3w.tile([1, TOK], F32, name="lbh",
                                            tag="lbh", bufs=2)
                            nc.sync.dma_start(
                                lbh[:], lB_d[h * TOK:(h + 1) * TOK]
                                .rearrange("(o t) -> o t", o=1))
                            lsb = ph3w.tile([1, TOK], F32, name="lsb",
                                            tag="lsb")
                            nc.vector.tensor_add(lsb[:], lps[:], lbh[:])
                            lrep = ps3.tile([128, TOK], F32, name="lrep",
                                            tag="lrep", bufs=1)
                            nc.tensor.matmul(lrep[:], ones_row[:], lsb[:],
                                             start=True, stop=True)
                            linv = ph3w.tile([128, TOK], F32, name="linv",
                                             tag="linv", bufs=2)
                            nc.vector.reciprocal(linv[:], lrep[:])
                            avf = ph3w.tile([128, TOK], F32, name="avf",
                                            tag="avf", bufs=2)
                            nc.vector.tensor_add(avf[:], avps[:], ctxB[h][:])
                            nc.vector.scalar_tensor_tensor(
                                ctxT[h // 2][:, h % 2, :], avf[:], SCTX,
                                linv[:], op0=ALU.mult, op1=ALU.mult)

            # ===== phase 4: Wo + residual -> x2
            OSCALE = 1.0 / (SCTX * SWD)
            with tc.tile_pool(name="ph4w", bufs=3) as ph4w, \
                 tc.tile_pool(name="ps4", bufs=2, space="PSUM") as ps4:
                for nd in range(NDC):
                    psl = [ps4.tile([128, DCH], F32, name=f"mm{t}",
                                    tag=f"mm{t}") for t in range(NT)]
                    for kg in range(KGD):
                        wt = wsp.tile([128, 2, DCH], F8, name="wt", tag="w")
                        base = (kg * NDC + nd) * 128
                        nc.sync.dma_start(
                            wt[:], wo[base:base + 128, :]
                            .rearrange("p (a b) -> p a b", a=2))
                        for t in range(NT):
                            nc.tensor.matmul(
                                psl[t][:],
                                ctxT[kg][:, :, t * 128:(t + 1) * 128],
                                wt[:], start=(kg == 0), stop=(kg == KGD - 1),
                                perf_mode=DR)
                    for t in range(NT):
                        xf = ph4w.tile([128, DCH], F32, name="xf", tag="xf")
                        nc.sync.dma_start(
                            xf[:], x_in[t * 128:(t + 1) * 128,
                                        nd * DCH:(nd + 1) * DCH])
                        tt1 = ph4w.tile([128, DCH], F32, name="tt1", tag="tt1")
                        nc.vector.scalar_tensor_tensor(
                            tt1[:], psl[t][:], OSCALE, xf[:],
                            op0=ALU.mult, op1=ALU.add)
                        nc.vector.tensor_add(
                            x2_t[t][:, nd * DCH:(nd + 1) * DCH], tt1[:],
                            bo_rep[:, nd * DCH:(nd + 1) * DCH])
                for t in range(NT):
                    nc.vector.tensor_reduce(sums_x2[t][:], x2_t[t][:],
                                            axis=AX.X, op=ALU.add)

        # ===== phases 5-7: LN, FFN, output
        with tc.tile_pool(name="ffnp", bufs=1) as ffnp:
            h2T = [ffnp.tile([128, 2, TOK], F8, name=f"h2T_{g}")
                   for g in range(KGD)]
            uT = [ffnp.tile([128, 2, TOK], F8, name=f"uT_{g}")
                  for g in range(KGF)]
            sT = [ffnp.tile([128, 2, TOK], F8, name=f"sT_{g}")
                  for g in range(KGF)]

            with tc.tile_pool(name="ph5w", bufs=2) as ph5w, \
                 tc.tile_pool(name="ps5", bufs=4, space="PSUM") as ps5:
                for t in range(NT):
                    nmu = ph5w.tile([128, 1], F32, name="nmu", tag="nmu")
                    nc.vector.tensor_scalar(nmu[:], sums_x2[t][:], -1.0 / D,
                                            None, op0=ALU.mult)
                    ss = ph5w.tile([128, NDC], F32, name="ss5", tag="ss5")
                    sq = ph5w.tile([128, DCH], F32, name="sq5", tag="sq5")
                    for c in range(NDC):
                        nc.scalar.activation(
                            sq[:], x2_t[t][:, c * DCH:(c + 1) * DCH],
                            AF.Square, bias=nmu[:], accum_out=ss[:, c:c + 1])
                    var = ph5w.tile([128, 1], F32, name="var", tag="var")
                    nc.vector.tensor_reduce(var[:], ss[:], axis=AX.X,
                                            op=ALU.add)
                    # sqrt((var+eps)/SH2^2) -> recip = SH2 * rsqrt(var+eps)
                    nc.vector.tensor_scalar(var[:], var[:],
                                            1.0 / (D * SH2 * SH2),
                                            LN_EPS / (SH2 * SH2),
                                            op0=ALU.mult, op1=ALU.add)
                    nc.scalar.sqrt(var[:], var[:])
                    rs = ph5w.tile([128, 1], F32, name="rs5", tag="rs5")
                    nc.vector.reciprocal(rs[:], var[:])
                    nrs = ph5w.tile([128, 1], F32, name="nrs", tag="nrs")
                    nc.vector.tensor_mul(nrs[:], nmu[:], rs[:])
                    h2 = ph5w.tile([128, D], BF16, name="h2", tag="h2")
                    nc.scalar.activation(h2[:], x2_t[t][:], AF.Identity,
                                         bias=nrs[:], scale=rs[:])
                    for k in range(KD):
                        tp = ps5.tile([128, 128], BF16, name="tp5", tag="tp5")
                        nc.tensor.transpose(tp[:], h2[:, k * 128:(k + 1) * 128],
                                            ident[:])
                        nc.scalar.copy(
                            h2T[k // 2][:, k % 2, t * 128:(t + 1) * 128],
                            tp[:])

            U_EVAC = SU / (SH2 * SWD)
            G_SCALE = 1.0 / (SU * SWF)
            with tc.tile_pool(name="ph6w", bufs=2) as ph6w, \
                 tc.tile_pool(name="ps6", bufs=2, space="PSUM") as ps6:
                for mb in range(KF // 4):
                    psl = [ps6.tile([128, TOK], F32, name=f"mm{m}",
                                    tag=f"mm{m}") for m in range(4)]
                    for kg in range(KGD):
                        wt = wsp.tile([128, 2, DCH], F8, name="wt", tag="w")
                        base = (kg * NFC + mb) * 128
                        nc.sync.dma_start(
                            wt[:], w1[base:base + 128, :]
                            .rearrange("p (a b) -> p a b", a=2))
                        for m in range(4):
                            nc.tensor.matmul(
                                psl[m][:], wt[:, :, m * 128:(m + 1) * 128],
                                h2T[kg][:], start=(kg == 0),
                                stop=(kg == KGD - 1), perf_mode=DR)
                    for m in range(4):
                        kf = mb * 4 + m
                        nc.scalar.activation(uT[kf // 2][:, kf % 2, :],
                                             psl[m][:], AF.Identity,
                                             bias=b1t[:, kf:kf + 1],
                                             scale=U_EVAC)

                for mb in range(KF // 4):
                    g1l = [ph6w.tile([128, TOK], BF16, name=f"g1_{m}",
                                     tag=f"g1_{m}") for m in range(4)]
                    psl = [ps6.tile([128, TOK], F32, name=f"mm{m}",
                                    tag=f"mm{m}") for m in range(4)]
                    for kg in range(KGF):
                        wt = wsp.tile([128, 2, DCH], F8, name="wt", tag="w")
                        base = (kg * NFC + mb) * 128
                        nc.sync.dma_start(
                            wt[:], wg1[base:base + 128, :]
                            .rearrange("p (a b) -> p a b", a=2))
                        for m in range(4):
                            nc.tensor.matmul(
                                psl[m][:], wt[:, :, m * 128:(m + 1) * 128],
                                uT[kg][:], start=(kg == 0),
                                stop=(kg == KGF - 1), perf_mode=DR)
                    for m in range(4):
                        kf = mb * 4 + m
                        sg = ph6w.tile([128, TOK], BF16, name="sg", tag="sg")
                        nc.scalar.activation(sg[:], psl[m][:], AF.Sigmoid,
                                             bias=bg1t[:, kf:kf + 1],
                                             scale=G_SCALE)
                        g1b = ph6w.tile([128, TOK], BF16, name="g1b",
                                        tag="g1b")
                        nc.scalar.activation(g1b[:], psl[m][:], AF.Identity,
                                             bias=bg1t[:, kf:kf + 1],
                                             scale=G_SCALE)
                        nc.vector.tensor_mul(g1l[m][:], sg[:], g1b[:])
                    psl2 = [ps6.tile([128, TOK], F32, name=f"mm{m}",
                                     tag=f"mm{m}") for m in range(4)]
                    for kg in range(KGF):
                        wt = wsp.tile([128, 2, DCH], F8, name="wt", tag="w")
                        base = (kg * NFC + mb) * 128
                        nc.sync.dma_start(
                            wt[:], wg2[base:base + 128, :]
                            .rearrange("p (a b) -> p a b", a=2))
                        for m in range(4):
                            nc.tensor.matmul(
                                psl2[m][:], wt[:, :, m * 128:(m + 1) * 128],
                                uT[kg][:], start=(kg == 0),
                                stop=(kg == KGF - 1), perf_mode=DR)
                    for m in range(4):
                        kf = mb * 4 + m
                        # side2 = SS * (g2 + bg2); sT = g1l * side2 (fp8)
                        side2 = ph6w.tile([128, TOK], BF16, name="side2",
                                          tag="side2")
                        nc.scalar.activation(side2[:], psl2[m][:], AF.Identity,
                                             bias=bg2t[:, kf:kf + 1],
                                             scale=SS * G_SCALE)
                        nc.vector.tensor_mul(sT[kf // 2][:, kf % 2, :],
                                             g1l[m][:], side2[:])

            Y_SCALE = 1.0 / (SS * SWF)
            with tc.tile_pool(name="ph7w", bufs=3) as ph7w, \
                 tc.tile_pool(name="ps7", bufs=2, space="PSUM") as ps7:
                for nd in range(NDC):
                    psl = [ps7.tile([128, DCH], F32, name=f"mm{t}",
                                    tag=f"mm{t}") for t in range(NT)]
                    for kg in range(KGF):
                        wt = wsp.tile([128, 2, DCH], F8, name="wt", tag="w")
                        base = (kg * NDC + nd) * 128
                        nc.sync.dma_start(
                            wt[:], w2[base:base + 128, :]
                            .rearrange("p (a b) -> p a b", a=2))
                        for t in range(NT):
                            nc.tensor.matmul(
                                psl[t][:],
                                sT[kg][:, :, t * 128:(t + 1) * 128],
                                wt[:], start=(kg == 0), stop=(kg == KGF - 1),
                                perf_mode=DR)
                    for t in range(NT):
                        tt1 = ph7w.tile([128, DCH], F32, name="o1", tag="o1")
                        nc.vector.scalar_tensor_tensor(
                            tt1[:], psl[t][:], Y_SCALE,
                            x2_t[t][:, nd * DCH:(nd + 1) * DCH],
                            op0=ALU.mult, op1=ALU.add)
                        yf = ph7w.tile([128, DCH], F32, name="yf", tag="yf")
                        nc.vector.tensor_add(
                            yf[:], tt1[:], b2_rep[:, nd * DCH:(nd + 1) * DCH])
                        nc.sync.dma_start(
                            out_d[t * 128:(t + 1) * 128,
                                  nd * DCH:(nd + 1) * DCH], yf[:])
    n = split_excess_waits(nc)
    return nc


# ---------------------------------------------------------------- host side


def pack_pair(W, s, och=512):
    """[K, N] f32 -> fp8 row-pair chunks [(KG*NCH)*128, 2*och]."""
    K, N = W.shape
    kg, nch = K // 256, max(N // och, 1)
    och = min(och, N)
    q = np.clip(np.asarray(W, np.float32) * s, -240.0, 240.0).astype(NP_F8)
    t = q.reshape(kg, 2, 128, nch, och).transpose(0, 3, 2, 1, 4)
    return np.ascontiguousarray(t.reshape(kg * nch * 128, 2 * och))


def host_prepare(inputs, cfg):
    B, T, D, H, DFF = cfg["B"], cfg["T"], cfg["D"], cfg["H"], cfg["DFF"]
    dv = derived(cfg)
    HD, TOK = dv["HD"], dv["TOK"]
    f32 = np.float32
    DCH = min(512, D)

    x = np.asarray(inputs["x"], f32)
    g_rms = np.asarray(inputs["g_rms"], f32)
    g_ln = np.asarray(inputs["g_ln"], f32)
    b_ln = np.asarray(inputs["b_ln"], f32)
    pad = np.asarray(inputs["pad_mask"])

    perm = np.concatenate(
        [h * HD + np.concatenate([np.arange(0, HD, 2), np.arange(1, HD, 2)])
         for h in range(H)])
    wq = pack_pair((g_rms[:, None] * np.asarray(inputs["Wq"], f32))[:, perm],
                   SWD, DCH)
    wk = pack_pair((g_rms[:, None] * np.asarray(inputs["Wk"], f32))[:, perm],
                   SWD, DCH)
    wv = pack_pair(g_rms[:, None] * np.asarray(inputs["Wv"], f32), SWD, DCH)
    wo = pack_pair(np.asarray(inputs["Wo"], f32), SWD, DCH)
    w1 = pack_pair(g_ln[:, None] * np.asarray(inputs["W1"], f32), SWD, DCH)
    wg1 = pack_pair(np.asarray(inputs["Wg1"], f32), SWF, DCH)
    wg2 = pack_pair(np.asarray(inputs["Wg2"], f32), SWF, DCH)
    w2 = pack_pair(np.asarray(inputs["W2"], f32), SWF, DCH)

    qscale = 1.0 / math.sqrt(HD)
    bqc = (np.asarray(inputs["bq"], f32)[perm] * qscale).astype(f32)
    bkp = np.asarray(inputs["bk"], f32)[perm].astype(f32)
    b1p8 = (np.asarray(inputs["b1"], f32)
            + b_ln @ np.asarray(inputs["W1"], f32)).astype(f32) * np.float32(SU)
    bg1 = np.asarray(inputs["bg1"], f32)
    bg28 = np.asarray(inputs["bg2"], f32)
    bo_rep = np.broadcast_to(np.asarray(inputs["bo"], f32), (128, D)).copy()
    b2_rep = np.broadcast_to(np.asarray(inputs["b2"], f32), (128, D)).copy()

    inv_freq = 1.0 / (10000.0 ** (np.arange(0, HD, 2, dtype=f32) / HD))
    ang = np.arange(T, dtype=f32)[:, None] * inv_freq[None, :]
    cosA, sinA = np.cos(ang).astype(f32), np.sin(ang).astype(f32)

    tri = np.where(np.arange(128)[:, None] <= np.arange(128)[None, :],
                   np.float32(0.0), np.float32(NEG))

    in_maps = []
    for i in range(CORES):
        g, p = i // GPC, i % GPC
        t0 = p * TOK
        kb = np.where(pad[g] == 0, np.float32(NEG), np.float32(0.0))
        kb[t0:] = NEG
        kbo = np.where(pad[g, t0:t0 + TOK] == 0, np.float32(NEG),
                       np.float32(0.0))
        in_maps.append(dict(
            x=np.ascontiguousarray(x[g, t0:t0 + TOK]),
            wq=wq, wk=wk, wv=wv, wo=wo, w1=w1, wg1=wg1, wg2=wg2, w2=w2,
            bqc=bqc, bkp=bkp, b1p8=b1p8, bg1=bg1, bg28=bg28,
            bo_rep=bo_rep, b2_rep=b2_rep,
            cosT=np.ascontiguousarray(
                np.tile(cosA[t0:t0 + TOK].T, (2, 1))),
            sinT=np.ascontiguousarray(
                np.tile(sinA[t0:t0 + TOK].T, (2, 1))),
            keybias=kb, keybias_own=kbo, triT=tri,
        ))
    return in_maps


def host_assemble(results, cfg):
    B, T, D = cfg["B"], cfg["T"], cfg["D"]
    TOK = derived(cfg)["TOK"]
    out = np.empty((B, T, D), np.float32)
    for i in range(CORES):
        g, p = i // GPC, i % GPC
        out[g, p * TOK:(p + 1) * TOK] = results[i]["out"]
    return out


# ---------------------------------------------------------------- numpy ref


def numpy_reference(inputs, cfg):
    B, T, D, H, DFF = cfg["B"], cfg["T"], cfg["D"], cfg["H"], cfg["DFF"]
    HD = D // H
    f = np.float32
    x = np.asarray(inputs["x"], f)
    RMS_EPS = float(np.finfo(np.float32).eps)

    h = x * (1.0 / np.sqrt((x * x).mean(-1, keepdims=True) + RMS_EPS))
    h = h * inputs["g_rms"]
    q = (h @ inputs["Wq"] + inputs["bq"]).reshape(B, T, H, HD).transpose(0, 2, 1, 3)
    k = (h @ inputs["Wk"] + inputs["bk"]).reshape(B, T, H, HD).transpose(0, 2, 1, 3)
    v = (h @ inputs["Wv"]).reshape(B, T, H, HD).transpose(0, 2, 1, 3)

    inv_freq = 1.0 / (10000.0 ** (np.arange(0, HD, 2, dtype=f) / HD))
    ang = np.arange(T, dtype=f)[:, None] * inv_freq[None, :]
    cos, sin = np.cos(ang), np.sin(ang)

    def rope(z):
        z1, z2 = z[..., ::2], z[..., 1::2]
        out = np.stack([z1 * cos - z2 * sin, z1 * sin + z2 * cos], -1)
        return out.reshape(z.shape)

    q, k = rope(q), rope(k)
    scores = np.einsum("bhqd,bhkd->bhqk", q, k) / np.sqrt(np.float32(HD))
    causal = np.tril(np.ones((T, T), bool))
    mask = (np.asarray(inputs["pad_mask"])[:, None, :].astype(bool)
            & causal)[:, None]
    scores = np.where(mask, scores, -np.inf)
    m = scores.max(-1, keepdims=True)
    e = np.exp(scores - m)
    attn = e / e.sum(-1, keepdims=True)
    o = np.einsum("bhqk,bhkd->bhqd", attn, v)
    o = o.transpose(0, 2, 1, 3).reshape(B, T, D)
    x = x + o @ inputs["Wo"] + inputs["bo"]

    mu = x.mean(-1, keepdims=True)
    var = ((x - mu) ** 2).mean(-1, keepdims=True)
    h2 = (x - mu) / np.sqrt(var + 1e-5) * inputs["g_ln"] + inputs["b_ln"]
    u = h2 @ inputs["W1"] + inputs["b1"]
    g1 = u @ inputs["Wg1"] + inputs["bg1"]
    s = (g1 / (1 + np.exp(-g1))) * (u @ inputs["Wg2"] + inputs["bg2"])
    return x + s @ inputs["W2"] + inputs["b2"]


def make_small_inputs(cfg, seed=0):
    B, T, D, H, DFF = cfg["B"], cfg["T"], cfg["D"], cfg["H"], cfg["DFF"]
    rng = np.random.default_rng(seed)
    f = np.float32

    def w(shape, fan):
        return ((rng.random(shape, dtype=f) * 2 - 1) / np.sqrt(fan)).astype(f)

    lengths = rng.integers(T // 2, T + 1, size=(B,))
    pad = (np.arange(T)[None, :] < lengths[:, None]).astype(np.int32)
    return dict(
        x=rng.standard_normal((B, T, D), dtype=f),
        Wq=w((D, D), D), bq=rng.standard_normal(D, dtype=f) * 0.02,
        Wk=w((D, D), D), bk=rng.standard_normal(D, dtype=f) * 0.02,
        Wv=w((D, D), D),
        Wo=w((D, D), D), bo=rng.standard_normal(D, dtype=f) * 0.02,
        W1=w((D, DFF), D), b1=rng.standard_normal(DFF, dtype=f) * 0.02,
        Wg1=w((DFF, DFF), DFF), bg1=rng.standard_normal(DFF, dtype=f) * 0.02,
        Wg2=w((DFF, DFF), DFF), bg2=rng.standard_normal(DFF, dtype=f) * 0.02,
        W2=w((DFF, D), DFF), b2=rng.standard_normal(D, dtype=f) * 0.02,
        g_rms=(1 + 0.1 * rng.standard_normal(D)).astype(f),
        g_ln=(1 + 0.1 * rng.standard_normal(D)).astype(f),
        b_ln=(0.05 * rng.standard_normal(D)).astype(f),
        pad_mask=pad,
    )


# ===================== tile scheduler patch =====================


import concourse.tile as tile


def _split_drain_and_barrier(self, tick_clock, wait_clock):
    from concourse.vector_clock import ScopedClock

    drain_inst = self.nc.sync.drain()
    wait_clock.add_sem_waits(
        drain_inst.ins, ScopedClock({None: tick_clock.global_clock})
    )
    si = drain_inst.ins.sync_info
    waits = list(si.on_wait) if si and si.on_wait else []
    if len(waits) > 1:
        si.on_wait.clear()
        si.on_wait.extend(waits[:1])
        for i in range(1, len(waits), 1):
            extra = self.nc.sync.drain()
            esi = extra.ins.sync_info
            if esi is None:
                import concourse.mybir as mybir

                extra.ins.sync_info = mybir.SyncInfo(
                    on_wait=waits[i : i + 1], on_update=[]
                )
            else:
                esi.on_wait.extend(waits[i : i + 1])

    self.nc.all_engine_barrier()
    assert self.sems is not None
    popped = self.nc._tile_sem_poison_stack.pop()
    assert popped is self._sem_poison
    self.nc.clear_and_free_semaphores(list(self.sems.allocated().values()))
    self.nc.all_engine_barrier()


def split_excess_waits(nc, default_limit=1, ctrl_limit=1, dma_limit=1):
    """Walrus in this container rejects instructions whose sync_info
    carries more wait commands than the ISA encoding has slots for.
    Move excess waits onto same-engine no-op carriers inserted right
    before the offending instruction (engine queues are in-order, so the
    carrier's waits are observed before the instruction issues)."""
    import concourse.mybir as mybir

    CTRL = ("InstDrain", "InstNoOp", "InstEventSemaphore")
    DMA = ("InstDMACopy", "InstTriggeredCopy", "InstDMATranspose")
    nsplit = 0
    for bb_name, bbw in list(nc.bb_map.items()):
        bb = bbw.bb if hasattr(bbw, "bb") else bbw
        insts = bb.instructions
        i = 0
        while i < len(insts):
            inst = insts[i]
            tname = type(inst).__name__
            limit = (ctrl_limit if tname in CTRL
                     else dma_limit if tname in DMA else default_limit)
            si = inst.sync_info
            waits = list(si.on_wait) if si and si.on_wait else []
            if len(waits) > limit:
                keep, extra = waits[:limit], waits[limit:]
                si.on_wait.clear()
                si.on_wait.extend(keep)
                ncar = 0
                for j in range(0, len(extra), ctrl_limit):
                    chunk = extra[j:j + ctrl_limit]
                    car = nc.engines[inst.engine].nop(nofuse=True).ins
                    # nop() appended to the current bb; move it here
                    for other in nc.bb_map.values():
                        obb = other.bb if hasattr(other, "bb") else other
                        if obb.instructions and obb.instructions[-1] is car:
                            obb.instructions.pop()
                            break
                    car.sync_info = mybir.SyncInfo(on_wait=chunk, on_update=[])
                    insts.insert(i, car)
                    ncar += 1
                i += ncar
                nsplit += 1
            i += 1
    return nsplit


def _apply_tile_patch():
    tile.TileContext._drain_and_barrier = _split_drain_and_barrier


# ================================================================ runner

_tile_patch_applied = False
_build_cache = {}
LAST_EXEC_NS = None


def _get_nc():
    global _tile_patch_applied
    if not _tile_patch_applied:
        _apply_tile_patch()
        _tile_patch_applied = True
    if "nc" not in _build_cache:
        nc = bass.Bass()
        build(nc, full_cfg())
        _build_cache["nc"] = nc
    return _build_cache["nc"]


def kernel(_profile=False, **inputs):
    """Full-input decoder block on 8 TRN2 NeuronCores.

    inputs: the arrays from reference.setup_inputs() (numpy or jax).
    Returns the full [B, T, D] float32 output.
    """
    global LAST_EXEC_NS
    from concourse.bass_utils import run_bass_kernel_spmd

    cfg = full_cfg()
    nc = _get_nc()
    in_maps = host_prepare({k: np.asarray(v) for k, v in inputs.items()}, cfg)
    res = run_bass_kernel_spmd(nc, in_maps, list(range(CORES)),
                               trace=bool(_profile))
    LAST_EXEC_NS = getattr(res, "exec_time_ns", None)
    return host_assemble(res.results, cfg)


# revision 25
# speedup vs baseline: 1.5894x; 1.0423x over previous
"""nn_DecoderBlock Trainium2 kernel — 8 NeuronCores, token-sharded.

Self-contained: builds a Bass/Tile SPMD program (one program, all 8
cores; per-core differences are input data), runs it via
run_bass_kernel_spmd, reassembles the full output on the host.

All 8 linear layers run in fp8(e4m3) with DoubleRow matmuls (K=256 per
instruction); attention QK/softmax/AV stays bf16/f32.
"""


import math
from contextlib import ExitStack

import numpy as np
import ml_dtypes

import concourse.bass as bass
import concourse.mybir as mybir
from concourse.tile import TileContext
from concourse.masks import make_identity

F32 = mybir.dt.float32
BF16 = mybir.dt.bfloat16
F8 = mybir.dt.float8e4
NP_F8 = ml_dtypes.float8_e4m3
AF = mybir.ActivationFunctionType
ALU = mybir.AluOpType
AX = mybir.AxisListType
DR = mybir.MatmulPerfMode.DoubleRow

NEG = -1.0e9
CORES = 8
GPC = 4

# fp8 scales (powers of two; folded out at PSUM evacuation)
SH = 8.0     # rms-normed h
SH2 = 8.0    # layernormed h2
SU = 8.0     # ffn mid u
SS = 8.0     # swiglu out s
SCTX = 16.0  # attention context
SWD = 32.0   # weights with fan-in D
SWF = 64.0   # weights with fan-in DFF
SQK = 32.0   # rope'd q and k (each; k*32 max ~108, q*qscale*32 max ~10)
SP = 16.0    # softmax numerator exp(score)  (max exp(score) ~8.5 -> 136<240)
SV = 8.0     # v


def full_cfg():
    return dict(B=2, T=2048, D=2048, H=16, DFF=4096)


def small_cfg():
    return dict(B=2, T=1024, D=512, H=4, DFF=1024)


def derived(cfg):
    B, T, D, H, DFF = cfg["B"], cfg["T"], cfg["D"], cfg["H"], cfg["DFF"]
    HD = D // H
    assert HD == 128
    TOK = B * T // CORES
    assert T // GPC == TOK and TOK % 128 == 0
    assert (TOK // 128) % 2 == 0  # kb pairing for DoubleRow AV
    return dict(HD=HD, TOK=TOK, NT=TOK // 128, KD=D // 128, KF=DFF // 128,
                NKB=T // 128, KGD=D // 256, KGF=DFF // 256)


def build(nc: bass.Bass, cfg):
    B, T, D, H, DFF = cfg["B"], cfg["T"], cfg["D"], cfg["H"], cfg["DFF"]
    dv = derived(cfg)
    TOK, NT, KD, KF, NKB = dv["TOK"], dv["NT"], dv["KD"], dv["KF"], dv["NKB"]
    KGD, KGF = dv["KGD"], dv["KGF"]
    DCH = min(512, D)
    NDC = D // DCH
    NFC = DFF // DCH
    RMS_EPS = float(np.finfo(np.float32).eps)
    LN_EPS = 1e-5
    DT = D * TOK

    x_in = nc.declare_dram_parameter("x", [TOK, D], F32, isOutput=False)
    # fp8 weights, packed as [(KG*NCH)*128, 2*512] row-pair chunks
    wq = nc.declare_dram_parameter("wq", [KGD * NDC * 128, 2 * DCH], F8, isOutput=False)
    wk = nc.declare_dram_parameter("wk", [KGD * NDC * 128, 2 * DCH], F8, isOutput=False)
    wv = nc.declare_dram_parameter("wv", [KGD * NDC * 128, 2 * DCH], F8, isOutput=False)
    wo = nc.declare_dram_parameter("wo", [KGD * NDC * 128, 2 * DCH], F8, isOutput=False)
    w1 = nc.declare_dram_parameter("w1", [KGD * NFC * 128, 2 * DCH], F8, isOutput=False)
    wg1 = nc.declare_dram_parameter("wg1", [KGF * NFC * 128, 2 * DCH], F8, isOutput=False)
    wg2 = nc.declare_dram_parameter("wg2", [KGF * NFC * 128, 2 * DCH], F8, isOutput=False)
    w2 = nc.declare_dram_parameter("w2", [KGF * NDC * 128, 2 * DCH], F8, isOutput=False)
    bqc_d = nc.declare_dram_parameter("bqc", [D], F32, isOutput=False)
    bkp_d = nc.declare_dram_parameter("bkp", [D], F32, isOutput=False)
    b1_d = nc.declare_dram_parameter("b1p8", [DFF], F32, isOutput=False)
    bg1_d = nc.declare_dram_parameter("bg1", [DFF], F32, isOutput=False)
    bg2_d = nc.declare_dram_parameter("bg28", [DFF], F32, isOutput=False)
    bo_rep_d = nc.declare_dram_parameter("bo_rep", [128, D], BF16, isOutput=False)
    b2_rep_d = nc.declare_dram_parameter("b2_rep", [128, D], BF16, isOutput=False)
    cos_d = nc.declare_dram_parameter("cosT", [128, TOK], F32, isOutput=False)
    sin_d = nc.declare_dram_parameter("sinT", [128, TOK], F32, isOutput=False)
    keybias_d = nc.declare_dram_parameter("keybias", [T], F32, isOutput=False)
    kbown_d = nc.declare_dram_parameter("keybias_own", [TOK], F32, isOutput=False)
    tri_d = nc.declare_dram_parameter("triT", [128, 128], F32, isOutput=False)
    out_d = nc.declare_dram_parameter("out", [TOK, D], F32, isOutput=True)

    with TileContext(nc) as tc, ExitStack() as top:
        constp = top.enter_context(tc.tile_pool(name="constp", bufs=1))
        dramp = top.enter_context(tc.tile_pool(name="dramp", bufs=1, space="DRAM"))
        wsp = top.enter_context(tc.tile_pool(name="wsp", bufs=16))
        x2p = top.enter_context(tc.tile_pool(name="x2p", bufs=1))

        # ---- constants
        ident = constp.tile([128, 128], BF16, name="ident")
        make_identity(nc, ident[:])
        # DoubleRow weight APs need the pair-halves >=16B apart
        ones8 = constp.tile([128, 2, 16], F8, name="ones8")
        nc.vector.memset(ones8[:], 1.0)
        ones_row = constp.tile([1, 128], F32, name="ones_row")
        nc.vector.memset(ones_row[:], 1.0)
        tri = constp.tile([128, 128], F32, name="tri")
        nc.sync.dma_start(tri[:], tri_d[:])
        cosT = constp.tile([128, TOK], F32, name="cosT")
        sinT = constp.tile([128, TOK], F32, name="sinT")
        nc.sync.dma_start(cosT[:], cos_d[:])
        nc.sync.dma_start(sinT[:], sin_d[:])
        kb_bias = constp.tile([128, NKB], F32, name="kb_bias")
        nc.sync.dma_start(kb_bias[:], keybias_d[:].rearrange("(n p) -> p n", p=128))
        kbo_bias = constp.tile([128, NT], F32, name="kbo_bias")
        nc.sync.dma_start(kbo_bias[:], kbown_d[:].rearrange("(n p) -> p n", p=128))
        bqc = constp.tile([128, KD], F32, name="bqc")
        nc.sync.dma_start(bqc[:], bqc_d[:].rearrange("(n p) -> p n", p=128))
        bkp = constp.tile([128, KD], F32, name="bkp")
        nc.sync.dma_start(bkp[:], bkp_d[:].rearrange("(n p) -> p n", p=128))
        b1t = constp.tile([128, KF], F32, name="b1t")
        nc.sync.dma_start(b1t[:], b1_d[:].rearrange("(n p) -> p n", p=128))
        bg1t = constp.tile([128, KF], F32, name="bg1t")
        nc.sync.dma_start(bg1t[:], bg1_d[:].rearrange("(n p) -> p n", p=128))
        bg2t = constp.tile([128, KF], F32, name="bg2t")
        nc.sync.dma_start(bg2t[:], bg2_d[:].rearrange("(n p) -> p n", p=128))
        bo_rep = constp.tile([128, D], BF16, name="bo_rep")
        nc.sync.dma_start(bo_rep[:], bo_rep_d[:])
        b2_rep = constp.tile([128, D], BF16, name="b2_rep")
        nc.sync.dma_start(b2_rep[:], b2_rep_d[:])

        # k (feature-major) then v (token-major) in one fp8 gather payload
        snd_kv = dramp.tile([2 * DT], F8, name="snd_kv")
        gat_kv = dramp.tile([GPC, 2 * DT], F8, name="gat_kv")

        # HAM warmup: tiny f32 matmuls anchored on freshly-written tiles keep
        # the PE activity monitor from re-throttling during low-matmul phases.
        def warm(pool, anchor_f32):
            c = min(32, int(anchor_f32.shape[-1]))
            wps = pool.tile([128, 512], F32, name="wps", tag="wps", bufs=2)
            nc.tensor.matmul(wps[:, 0:c], tri[:], anchor_f32[:, 0:c],
                             start=True, stop=True)

        x2_t = [x2p.tile([128, D], F32, name=f"x2_{t}") for t in range(NT)]
        sums_x2 = [x2p.tile([128, 1], F32, name=f"sx2_{t}") for t in range(NT)]
        ssq_x2 = [x2p.tile([128, NDC], F32, name=f"qx2_{t}") for t in range(NT)]

        with tc.tile_pool(name="ctxp", bufs=1) as ctxp:
            ctxT = [ctxp.tile([128, 2, TOK], F8, name=f"ctxT_{g}")
                    for g in range(KGD)]

            with tc.tile_pool(name="hTp", bufs=1) as hTp:
                hT = [hTp.tile([128, 2, TOK], F8, name=f"hT_{g}")
                      for g in range(KGD)]

                # ===== phase 1: RMSNorm + transpose -> hT (fp8, x SH)
                with tc.tile_pool(name="ph1w", bufs=2) as ph1w, \
                     tc.tile_pool(name="ps1", bufs=4, space="PSUM") as ps1:
                    for t in range(NT):
                        xt = ph1w.tile([128, D], F32, name="xt", tag="xt")
                        nc.sync.dma_start(xt[:], x_in[t * 128:(t + 1) * 128, :])
                        ss = ph1w.tile([128, NDC], F32, name="ss", tag="ss")
                        sq = ph1w.tile([128, DCH], F32, name="sq", tag="sq")
                        for c in range(NDC):
                            nc.vector.scalar_tensor_tensor(
                                sq[:], xt[:, c * DCH:(c + 1) * DCH], 1.0,
                                xt[:, c * DCH:(c + 1) * DCH],
                                op0=ALU.mult, op1=ALU.mult,
                                accum_out=ss[:, c:c + 1])
                            warm(ps1, ss[:, c:c + 1])
                        ssum = ph1w.tile([128, 1], F32, name="ssum", tag="ssum")
                        nc.vector.tensor_reduce(ssum[:], ss[:], axis=AX.X,
                                                op=ALU.add)
                        # sqrt((mean+eps)/SH^2) -> recip = SH * rsqrt(mean+eps)
                        nc.vector.tensor_scalar(
                            ssum[:], ssum[:], 1.0 / (D * SH * SH),
                            RMS_EPS / (SH * SH), op0=ALU.mult, op1=ALU.add)
                        nc.scalar.sqrt(ssum[:], ssum[:])
                        rs = ph1w.tile([128, 1], F32, name="rs", tag="rs")
                        nc.vector.reciprocal(rs[:], ssum[:])
                        warm(ps1, rs)
                        hn = ph1w.tile([128, D], BF16, name="hn",
                                       tag="hn", bufs=2)
                        nc.scalar.activation(hn[:], xt[:], AF.Copy, scale=rs[:])
                        for k in range(KD):
                            tp = ps1.tile([128, 128], BF16, name="tp", tag="tp")
                            nc.tensor.transpose(
                                tp[:], hn[:, k * 128:(k + 1) * 128], ident[:])
                            nc.scalar.copy(
                                hT[k // 2][:, k % 2, t * 128:(t + 1) * 128],
                                tp[:])

                with tc.tile_pool(name="qkvp", bufs=1) as qkvp:
                    qrT = [qkvp.tile([128, TOK], F8, name=f"qrT_{k}")
                           for k in range(KD)]
                    krT = [qkvp.tile([128, TOK], F8, name=f"krT_{k}")
                           for k in range(KD)]
                    vt_all = qkvp.tile([128, NT, D], F8, name="vt_all")

                    # ===== phase 2: projections + rope + send + gather
                    with tc.tile_pool(name="ph2w", bufs=4) as ph2w, \
                         tc.tile_pool(name="ps2", bufs=2, space="PSUM") as ps2:

                        def rope(dst, src):
                            # walrus: SB+SB tensor_tensor operands must share
                            # base partition -> cos/sin are replicated on both
                            # halves and tmps live at base 0
                            t1 = ph2w.tile([64, TOK], F32, name="rp1", tag="rp1")
                            t2 = ph2w.tile([64, TOK], F32, name="rp2", tag="rp2")
                            t3 = ph2w.tile([64, TOK], F32, name="rp3", tag="rp3")
                            t4 = ph2w.tile([64, TOK], F32, name="rp4", tag="rp4")
                            nc.vector.tensor_mul(t1[:], src[0:64, :], cosT[0:64, :])
                            nc.vector.tensor_mul(t2[:], src[64:128, :], sinT[64:128, :])
                            nc.vector.tensor_sub(dst[0:64, :], t1[:], t2[:])
                            nc.vector.tensor_mul(t3[:], src[0:64, :], sinT[0:64, :])
                            nc.vector.tensor_mul(t4[:], src[64:128, :], cosT[64:128, :])
                            nc.vector.tensor_add(dst[64:128, :], t3[:], t4[:])

                        qscale = 1.0 / math.sqrt(128.0)
                        PSCALE = 1.0 / (SH * SWD)

                        def proj_fmajor(wten, bias_t, scale_, dstl, send):
                            for mb in range(KD // 4):
                                psl = [ps2.tile([128, TOK], F32, name=f"mm{m}",
                                                tag=f"mm{m}") for m in range(4)]
                                for kg in range(KGD):
                                    wt = wsp.tile([128, 2, DCH], F8, name="wt",
                                                  tag="w")
                                    base = (kg * NDC + mb) * 128
                                    nc.sync.dma_start(
                                        wt[:], wten[base:base + 128, :]
                                        .rearrange("p (a b) -> p a b", a=2))
                                    for m in range(4):
                                        nc.tensor.matmul(
                                            psl[m][:],
                                            wt[:, :, m * 128:(m + 1) * 128],
                                            hT[kg][:], start=(kg == 0),
                                            stop=(kg == KGD - 1),
                                            perf_mode=DR)
                                for m in range(4):
                                    kd = mb * 4 + m
                                    raw = ph2w.tile([128, TOK], BF16,
                                                    name="rawqk", tag="rawqk")
                                    nc.scalar.activation(
                                        raw[:], psl[m][:], AF.Identity,
                                        bias=bias_t[:, kd:kd + 1],
                                        scale=scale_ * PSCALE)
                                    rope(dstl[kd][:], raw[:])
                                    if send:
                                        nc.sync.dma_start(
                                            snd_kv[kd * 128 * TOK:
                                                   (kd + 1) * 128 * TOK]
                                            .rearrange("(p t) -> p t", t=TOK),
                                            dstl[kd][:])

                        # k first (scaled xSQK; bias pre-scaled on host)
                        proj_fmajor(wk, bkp, SQK, krT, True)

                        # v token-major (t outer: consecutive matmuls share
                        # the stationary activation tile)
                        vdst = snd_kv[DT:2 * DT].rearrange(
                            "(a p d) -> p a d", p=128, d=D)
                        for t in range(NT):
                            psl = [ps2.tile([128, DCH], F32, name=f"mm{nd}",
                                            tag=f"mm{nd}") for nd in range(NDC)]
                            for kg in range(KGD):
                                wts = []
                                for nd in range(NDC):
                                    wt = wsp.tile([128, 2, DCH], F8, name="wt",
                                                  tag="w")
                                    base = (kg * NDC + nd) * 128
                                    nc.sync.dma_start(
                                        wt[:], wv[base:base + 128, :]
                                        .rearrange("p (a b) -> p a b", a=2))
                                    wts.append(wt)
                                for nd in range(NDC):
                                    nc.tensor.matmul(
                                        psl[nd][:],
                                        hT[kg][:, :, t * 128:(t + 1) * 128],
                                        wts[nd][:],
                                        start=(kg == 0), stop=(kg == KGD - 1),
                                        perf_mode=DR)
                            for nd in range(NDC):
                                nc.scalar.activation(
                                    vt_all[:, t, nd * DCH:(nd + 1) * DCH],
                                    psl[nd][:], AF.Copy, scale=SV * PSCALE)
                            nc.sync.dma_start(vdst[:, t, :], vt_all[:, t, :])

                        # one combined fp8 k+v gather
                        nc.gpsimd.collective_compute(
                            "AllGather", ALU.bypass,
                            replica_groups=[[0, 1, 2, 3], [4, 5, 6, 7]],
                            ins=[snd_kv[:]], outs=[gat_kv[:]])

                        # q last: overlaps the gather
                        proj_fmajor(wq, bqc, qscale * SQK, qrT, False)

                    # ===== phase 3: attention
                    # part B (the core's own causal diagonal) runs for ALL
                    # heads first -- it needs no gathered data, so it
                    # overlaps the k/v AllGathers; per-head partial
                    # (sum p*v, sum p) pairs are combined with part A after
                    # the gathers land.
                    with tc.tile_pool(name="ph3b", bufs=1) as ph3b, \
                         tc.tile_pool(name="ph3w", bufs=3) as ph3w, \
                         tc.tile_pool(name="ps3", bufs=1, space="PSUM") as ps3:
                        ctxB = [ph3b.tile([128, TOK], BF16, name=f"ctxB_{h}")
                                for h in range(H)]
                        lB_d = dramp.tile([H * TOK], F32, name="lB_d")
                        # gathered k (feature-major) and v (token-major)
                        # chunks, loaded whole right after the gather
                        kcs = [ph3b.tile([128, KD, TOK], F8, name=f"kc_{j}")
                               for j in range(GPC - 1)]
                        vcs = [ph3b.tile([128, NT, D], F8, name=f"vc_{j}")
                               for j in range(GPC - 1)]
                        for j in range(GPC - 1):
                            nc.sync.dma_start(
                                kcs[j][:],
                                gat_kv[j, 0:DT].rearrange(
                                    "(a p t) -> p a t", p=128, t=TOK))
                            nc.sync.dma_start(
                                vcs[j][:],
                                gat_kv[j, DT:2 * DT].rearrange(
                                    "(a p d) -> p a d", p=128, d=D))

                        ESC = 1.0 / (SQK * SQK)

                        def qk_av_pair(h, avps, lps, k_src, v_pair, bias_col,
                                       first, last, diag0):
                            # two 128-key blocks -> one DoubleRow AV/lps
                            sps = ps3.tile([128, 2 * TOK], F32, name="sps",
                                           tag="sps", bufs=2)
                            p2 = ph3w.tile([128, 2, TOK], F8, name="p", tag="p")
                            for i in range(2):
                                nc.tensor.matmul(
                                    sps[:, i * TOK:(i + 1) * TOK],
                                    k_src[i], qrT[h][:],
                                    start=True, stop=True)
                                if diag0 is not None:
                                    dg = diag0 + i
                                    nc.vector.tensor_add(
                                        sps[:, i * TOK + dg * 128:
                                            i * TOK + (dg + 1) * 128],
                                        sps[:, i * TOK + dg * 128:
                                            i * TOK + (dg + 1) * 128],
                                        tri[:])
                                nc.scalar.activation(
                                    p2[:, i, :], sps[:, i * TOK:(i + 1) * TOK],
                                    AF.Exp, bias=bias_col[i], scale=ESC)
                                if diag0 is not None and diag0 + i > 0:
                                    nc.vector.memset(
                                        p2[:, i, 0:(diag0 + i) * 128], 0.0)
                            nc.tensor.matmul(lps[:], ones8[:, :, 0:1], p2[:],
                                             start=first, stop=last,
                                             perf_mode=DR)
                            nc.tensor.matmul(avps[:], v_pair, p2[:],
                                             start=first, stop=last,
                                             perf_mode=DR)

                        NPB = NT // 2
                        for h in range(H):
                            avpsB = ps3.tile([128, TOK], F32, name="avps",
                                             tag="avps", bufs=2)
                            lpsB = ps3.tile([1, TOK], F32, name="lps",
                                            tag="lps", bufs=1)
                            for pb in range(NPB):
                                kbl = 2 * pb
                                qk_av_pair(
                                    h, avpsB, lpsB,
                                    [krT[h][:, kbl * 128:(kbl + 1) * 128],
                                     krT[h][:, (kbl + 1) * 128:(kbl + 2) * 128]],
                                    vt_all[:, kbl:kbl + 2,
                                           h * 128:(h + 1) * 128],
                                    [kbo_bias[:, kbl:kbl + 1],
                                     kbo_bias[:, kbl + 1:kbl + 2]],
                                    pb == 0, pb == NPB - 1, kbl)
                            nc.scalar.activation(ctxB[h][:], avpsB[:], AF.Copy,
                                                 scale=1.0 / (SP * SV))
                            ltmp = ph3w.tile([1, TOK], F32, name="ltmp",
                                             tag="ltmp", bufs=2)
                            nc.scalar.activation(ltmp[:], lpsB[:], AF.Copy,
                                                 scale=1.0 / SP)
                            nc.sync.dma_start(
                                lB_d[h * TOK:(h + 1) * TOK]
                                .rearrange("(o t) -> o t", o=1), ltmp[:])

                        NPA = (GPC - 1) * NPB
                        for h in range(H):
                            avps = ps3.tile([128, TOK], F32, name="avps",
                                            tag="avps", bufs=2)
                            lps = ps3.tile([1, TOK], F32, name="lps",
                                           tag="lps", bufs=1)
                            for j in range(GPC - 1):
                                for pb in range(NPB):
                                    kbl = 2 * pb
                                    kb = j * NT + kbl
                                    pi = j * NPB + pb
                                    qk_av_pair(
                                        h, avps, lps,
                                        [kcs[j][:, h, kbl * 128:(kbl + 1) * 128],
                                         kcs[j][:, h,
                                                (kbl + 1) * 128:(kbl + 2) * 128]],
                                        vcs[j][:, kbl:kbl + 2,
                                               h * 128:(h + 1) * 128],
                                        [kb_bias[:, kb:kb + 1],
                                         kb_bias[:, kb + 1:kb + 2]],
                                        pi == 0, pi == NPA - 1, None)

                            lbh = ph3w.tile([1, TOK], F32, name="lbh",
                                            tag="lbh", bufs=2)
                            nc.sync.dma_start(
                                lbh[:], lB_d[h * TOK:(h + 1) * TOK]
                                .rearrange("(o t) -> o t", o=1))
                            lsb = ph3w.tile([1, TOK], F32, name="lsb",
                                            tag="lsb")
                            nc.vector.scalar_tensor_tensor(
                                lsb[:], lps[:], 1.0 / SP, lbh[:],
                                op0=ALU.mult, op1=ALU.add)
                            lrep = ps3.tile([128, TOK], F32, name="lrep",
                                            tag="lrep", bufs=1)
                            nc.tensor.matmul(lrep[:], ones_row[:], lsb[:],
                                             start=True, stop=True)
                            linv = ph3w.tile([128, TOK], F32, name="linv",
                                             tag="linv", bufs=2)
                            nc.vector.reciprocal(linv[:], lrep[:])
                            avf = ph3w.tile([128, TOK], F32, name="avf",
                                            tag="avf", bufs=2)
                            nc.vector.scalar_tensor_tensor(
                                avf[:], avps[:], 1.0 / (SP * SV), ctxB[h][:],
                                op0=ALU.mult, op1=ALU.add)
                            nc.vector.scalar_tensor_tensor(
                                ctxT[h // 2][:, h % 2, :], avf[:], SCTX,
                                linv[:], op0=ALU.mult, op1=ALU.mult)

            # ===== phase 4: Wo + residual -> x2
            OSCALE = 1.0 / (SCTX * SWD)
            with tc.tile_pool(name="ph4w", bufs=3) as ph4w, \
                 tc.tile_pool(name="ps4", bufs=2, space="PSUM") as ps4:
                for nd in range(NDC):
                    psl = [ps4.tile([128, DCH], F32, name=f"mm{t}",
                                    tag=f"mm{t}") for t in range(NT)]
                    for kg in range(KGD):
                        wt = wsp.tile([128, 2, DCH], F8, name="wt", tag="w")
                        base = (kg * NDC + nd) * 128
                        nc.sync.dma_start(
                            wt[:], wo[base:base + 128, :]
                            .rearrange("p (a b) -> p a b", a=2))
                        for t in range(NT):
                            nc.tensor.matmul(
                                psl[t][:],
                                ctxT[kg][:, :, t * 128:(t + 1) * 128],
                                wt[:], start=(kg == 0), stop=(kg == KGD - 1),
                                perf_mode=DR)
                    for t in range(NT):
                        xf = ph4w.tile([128, DCH], F32, name="xf", tag="xf")
                        nc.sync.dma_start(
                            xf[:], x_in[t * 128:(t + 1) * 128,
                                        nd * DCH:(nd + 1) * DCH])
                        tt1 = ph4w.tile([128, DCH], F32, name="tt1", tag="tt1")
                        nc.vector.scalar_tensor_tensor(
                            tt1[:], psl[t][:], OSCALE, xf[:],
                            op0=ALU.mult, op1=ALU.add)
                        nc.vector.tensor_add(
                            x2_t[t][:, nd * DCH:(nd + 1) * DCH], tt1[:],
                            bo_rep[:, nd * DCH:(nd + 1) * DCH])
                        sqj = ph4w.tile([128, DCH], F32, name="sqj", tag="sqj")
                        nc.vector.scalar_tensor_tensor(
                            sqj[:], x2_t[t][:, nd * DCH:(nd + 1) * DCH], 1.0,
                            x2_t[t][:, nd * DCH:(nd + 1) * DCH],
                            op0=ALU.mult, op1=ALU.mult,
                            accum_out=ssq_x2[t][:, nd:nd + 1])
                for t in range(NT):
                    nc.vector.tensor_reduce(sums_x2[t][:], x2_t[t][:],
                                            axis=AX.X, op=ALU.add)

        # ===== phases 5-7: LN, FFN, output
        with tc.tile_pool(name="ffnp", bufs=1) as ffnp:
            h2T = [ffnp.tile([128, 2, TOK], F8, name=f"h2T_{g}")
                   for g in range(KGD)]
            uT = [ffnp.tile([128, 2, TOK], F8, name=f"uT_{g}")
                  for g in range(KGF)]
            sT = [ffnp.tile([128, 2, TOK], F8, name=f"sT_{g}")
                  for g in range(KGF)]

            with tc.tile_pool(name="ph5w", bufs=2) as ph5w, \
                 tc.tile_pool(name="ps5", bufs=4, space="PSUM") as ps5:
                for t in range(NT):
                    nmu = ph5w.tile([128, 1], F32, name="nmu", tag="nmu")
                    nc.vector.tensor_scalar(nmu[:], sums_x2[t][:], -1.0 / D,
                                            None, op0=ALU.mult)
                    musq = ph5w.tile([128, 1], F32, name="musq", tag="musq")
                    nc.vector.tensor_mul(musq[:], nmu[:], nmu[:])
                    sumsq = ph5w.tile([128, 1], F32, name="sumsq", tag="sumsq")
                    nc.vector.tensor_reduce(sumsq[:], ssq_x2[t][:], axis=AX.X,
                                            op=ALU.add)
                    # var = E[x^2] - mu^2
                    var = ph5w.tile([128, 1], F32, name="var", tag="var")
                    nc.vector.scalar_tensor_tensor(
                        var[:], sumsq[:], 1.0 / D, musq[:],
                        op0=ALU.mult, op1=ALU.subtract)
                    warm(ps5, var)
                    # sqrt((var+eps)/SH2^2) -> recip = SH2 * rsqrt(var+eps)
                    nc.vector.tensor_scalar(var[:], var[:],
                                            1.0 / (SH2 * SH2),
                                            LN_EPS / (SH2 * SH2),
                                            op0=ALU.mult, op1=ALU.add)
                    nc.scalar.sqrt(var[:], var[:])
                    rs = ph5w.tile([128, 1], F32, name="rs5", tag="rs5")
                    nc.vector.reciprocal(rs[:], var[:])
                    nrs = ph5w.tile([128, 1], F32, name="nrs", tag="nrs")
                    nc.vector.tensor_mul(nrs[:], nmu[:], rs[:])
                    h2 = ph5w.tile([128, D], BF16, name="h2", tag="h2")
                    nc.scalar.activation(h2[:], x2_t[t][:], AF.Identity,
                                         bias=nrs[:], scale=rs[:])
                    for k in range(KD):
                        tp = ps5.tile([128, 128], BF16, name="tp5", tag="tp5")
                        nc.tensor.transpose(tp[:], h2[:, k * 128:(k + 1) * 128],
                                            ident[:])
                        nc.scalar.copy(
                            h2T[k // 2][:, k % 2, t * 128:(t + 1) * 128],
                            tp[:])

            U_EVAC = SU / (SH2 * SWD)
            G_SCALE = 1.0 / (SU * SWF)
            with tc.tile_pool(name="ph6w", bufs=2) as ph6w, \
                 tc.tile_pool(name="ps6", bufs=2, space="PSUM") as ps6:
                for mb in range(KF // 4):
                    psl = [ps6.tile([128, TOK], F32, name=f"mm{m}",
                                    tag=f"mm{m}") for m in range(4)]
                    for kg in range(KGD):
                        wt = wsp.tile([128, 2, DCH], F8, name="wt", tag="w")
                        base = (kg * NFC + mb) * 128
                        nc.sync.dma_start(
                            wt[:], w1[base:base + 128, :]
                            .rearrange("p (a b) -> p a b", a=2))
                        for m in range(4):
                            nc.tensor.matmul(
                                psl[m][:], wt[:, :, m * 128:(m + 1) * 128],
                                h2T[kg][:], start=(kg == 0),
                                stop=(kg == KGD - 1), perf_mode=DR)
                    for m in range(4):
                        kf = mb * 4 + m
                        nc.scalar.activation(uT[kf // 2][:, kf % 2, :],
                                             psl[m][:], AF.Identity,
                                             bias=b1t[:, kf:kf + 1],
                                             scale=U_EVAC)

                for mb in range(KF // 4):
                    g1l = [ph6w.tile([128, TOK], BF16, name=f"g1_{m}",
                                     tag=f"g1_{m}") for m in range(4)]
                    psl = [ps6.tile([128, TOK], F32, name=f"mm{m}",
                                    tag=f"mm{m}") for m in range(4)]
                    for kg in range(KGF):
                        wt = wsp.tile([128, 2, DCH], F8, name="wt", tag="w")
                        base = (kg * NFC + mb) * 128
                        nc.sync.dma_start(
                            wt[:], wg1[base:base + 128, :]
                            .rearrange("p (a b) -> p a b", a=2))
                        for m in range(4):
                            nc.tensor.matmul(
                                psl[m][:], wt[:, :, m * 128:(m + 1) * 128],
                                uT[kg][:], start=(kg == 0),
                                stop=(kg == KGF - 1), perf_mode=DR)
                    for m in range(4):
                        kf = mb * 4 + m
                        sg = ph6w.tile([128, TOK], BF16, name="sg", tag="sg")
                        nc.scalar.activation(sg[:], psl[m][:], AF.Sigmoid,
                                             bias=bg1t[:, kf:kf + 1],
                                             scale=G_SCALE)
                        g1b = ph6w.tile([128, TOK], BF16, name="g1b",
                                        tag="g1b")
                        nc.scalar.activation(g1b[:], psl[m][:], AF.Identity,
                                             bias=bg1t[:, kf:kf + 1],
                                             scale=G_SCALE)
                        nc.vector.tensor_mul(g1l[m][:], sg[:], g1b[:])
                    psl2 = [ps6.tile([128, TOK], F32, name=f"mm{m}",
                                     tag=f"mm{m}") for m in range(4)]
                    for kg in range(KGF):
                        wt = wsp.tile([128, 2, DCH], F8, name="wt", tag="w")
                        base = (kg * NFC + mb) * 128
                        nc.sync.dma_start(
                            wt[:], wg2[base:base + 128, :]
                            .rearrange("p (a b) -> p a b", a=2))
                        for m in range(4):
                            nc.tensor.matmul(
                                psl2[m][:], wt[:, :, m * 128:(m + 1) * 128],
                                uT[kg][:], start=(kg == 0),
                                stop=(kg == KGF - 1), perf_mode=DR)
                    for m in range(4):
                        kf = mb * 4 + m
                        # side2 = SS * (g2 + bg2); sT = g1l * side2 (fp8)
                        side2 = ph6w.tile([128, TOK], BF16, name="side2",
                                          tag="side2")
                        nc.scalar.activation(side2[:], psl2[m][:], AF.Identity,
                                             bias=bg2t[:, kf:kf + 1],
                                             scale=SS * G_SCALE)
                        nc.vector.tensor_mul(sT[kf // 2][:, kf % 2, :],
                                             g1l[m][:], side2[:])

            Y_SCALE = 1.0 / (SS * SWF)
            with tc.tile_pool(name="ph7w", bufs=3) as ph7w, \
                 tc.tile_pool(name="ps7", bufs=2, space="PSUM") as ps7:
                for nd in range(NDC):
                    psl = [ps7.tile([128, DCH], F32, name=f"mm{t}",
                                    tag=f"mm{t}") for t in range(NT)]
                    for kg in range(KGF):
                        wt = wsp.tile([128, 2, DCH], F8, name="wt", tag="w")
                        base = (kg * NDC + nd) * 128
                        nc.sync.dma_start(
                            wt[:], w2[base:base + 128, :]
                            .rearrange("p (a b) -> p a b", a=2))
                        for t in range(NT):
                            nc.tensor.matmul(
                                psl[t][:],
                                sT[kg][:, :, t * 128:(t + 1) * 128],
                                wt[:], start=(kg == 0), stop=(kg == KGF - 1),
                                perf_mode=DR)
                    for t in range(NT):
                        tt1 = ph7w.tile([128, DCH], F32, name="o1", tag="o1")
                        nc.vector.scalar_tensor_tensor(
                            tt1[:], psl[t][:], Y_SCALE,
                            x2_t[t][:, nd * DCH:(nd + 1) * DCH],
                            op0=ALU.mult, op1=ALU.add)
                        yf = ph7w.tile([128, DCH], F32, name="yf", tag="yf")
                        nc.vector.tensor_add(
                            yf[:], tt1[:], b2_rep[:, nd * DCH:(nd + 1) * DCH])
                        nc.sync.dma_start(
                            out_d[t * 128:(t + 1) * 128,
                                  nd * DCH:(nd + 1) * DCH], yf[:])
    n = split_excess_waits(nc)
    return nc


# ---------------------------------------------------------------- host side


def pack_pair(W, s, och=512):
    """[K, N] f32 -> fp8 row-pair chunks [(KG*NCH)*128, 2*och]."""
    K, N = W.shape
    kg, nch = K // 256, max(N // och, 1)
    och = min(och, N)
    q = np.clip(np.asarray(W, np.float32) * s, -240.0, 240.0).astype(NP_F8)
    t = q.reshape(kg, 2, 128, nch, och).transpose(0, 3, 2, 1, 4)
    return np.ascontiguousarray(t.reshape(kg * nch * 128, 2 * och))


def host_prepare(inputs, cfg):
    B, T, D, H, DFF = cfg["B"], cfg["T"], cfg["D"], cfg["H"], cfg["DFF"]
    dv = derived(cfg)
    HD, TOK = dv["HD"], dv["TOK"]
    f32 = np.float32
    DCH = min(512, D)

    x = np.asarray(inputs["x"], f32)
    g_rms = np.asarray(inputs["g_rms"], f32)
    g_ln = np.asarray(inputs["g_ln"], f32)
    b_ln = np.asarray(inputs["b_ln"], f32)
    pad = np.asarray(inputs["pad_mask"])

    perm = np.concatenate(
        [h * HD + np.concatenate([np.arange(0, HD, 2), np.arange(1, HD, 2)])
         for h in range(H)])
    wq = pack_pair((g_rms[:, None] * np.asarray(inputs["Wq"], f32))[:, perm],
                   SWD, DCH)
    wk = pack_pair((g_rms[:, None] * np.asarray(inputs["Wk"], f32))[:, perm],
                   SWD, DCH)
    wv = pack_pair(g_rms[:, None] * np.asarray(inputs["Wv"], f32), SWD, DCH)
    wo = pack_pair(np.asarray(inputs["Wo"], f32), SWD, DCH)
    w1 = pack_pair(g_ln[:, None] * np.asarray(inputs["W1"], f32), SWD, DCH)
    wg1 = pack_pair(np.asarray(inputs["Wg1"], f32), SWF, DCH)
    wg2 = pack_pair(np.asarray(inputs["Wg2"], f32), SWF, DCH)
    w2 = pack_pair(np.asarray(inputs["W2"], f32), SWF, DCH)

    qscale = 1.0 / math.sqrt(HD)
    bqc = (np.asarray(inputs["bq"], f32)[perm] * qscale).astype(f32)
    bkp = np.asarray(inputs["bk"], f32)[perm].astype(f32)
    b1p8 = (np.asarray(inputs["b1"], f32)
            + b_ln @ np.asarray(inputs["W1"], f32)).astype(f32) * np.float32(SU)
    bg1 = np.asarray(inputs["bg1"], f32)
    bg28 = np.asarray(inputs["bg2"], f32)
    bf = ml_dtypes.bfloat16
    bo_rep = np.broadcast_to(np.asarray(inputs["bo"], bf), (128, D)).copy()
    b2_rep = np.broadcast_to(np.asarray(inputs["b2"], bf), (128, D)).copy()

    inv_freq = 1.0 / (10000.0 ** (np.arange(0, HD, 2, dtype=f32) / HD))
    ang = np.arange(T, dtype=f32)[:, None] * inv_freq[None, :]
    cosA, sinA = np.cos(ang).astype(f32), np.sin(ang).astype(f32)

    tri = np.where(np.arange(128)[:, None] <= np.arange(128)[None, :],
                   np.float32(0.0), np.float32(NEG))

    lnp = np.float32(math.log(SP))
    in_maps = []
    for i in range(CORES):
        g, p = i // GPC, i % GPC
        t0 = p * TOK
        kb = np.where(pad[g] == 0, np.float32(NEG), lnp)
        kb[t0:] = NEG
        kbo = np.where(pad[g, t0:t0 + TOK] == 0, np.float32(NEG), lnp)
        in_maps.append(dict(
            x=np.ascontiguousarray(x[g, t0:t0 + TOK]),
            wq=wq, wk=wk, wv=wv, wo=wo, w1=w1, wg1=wg1, wg2=wg2, w2=w2,
            bqc=bqc, bkp=bkp, b1p8=b1p8, bg1=bg1, bg28=bg28,
            bo_rep=bo_rep, b2_rep=b2_rep,
            cosT=np.ascontiguousarray(
                np.tile(cosA[t0:t0 + TOK].T, (2, 1))),
            sinT=np.ascontiguousarray(
                np.tile(sinA[t0:t0 + TOK].T, (2, 1))),
            keybias=kb, keybias_own=kbo, triT=tri,
        ))
    return in_maps


def host_assemble(results, cfg):
    B, T, D = cfg["B"], cfg["T"], cfg["D"]
    TOK = derived(cfg)["TOK"]
    out = np.empty((B, T, D), np.float32)
    for i in range(CORES):
        g, p = i // GPC, i % GPC
        out[g, p * TOK:(p + 1) * TOK] = results[i]["out"]
    return out


# ---------------------------------------------------------------- numpy ref


def numpy_reference(inputs, cfg):
    B, T, D, H, DFF = cfg["B"], cfg["T"], cfg["D"], cfg["H"], cfg["DFF"]
    HD = D // H
    f = np.float32
    x = np.asarray(inputs["x"], f)
    RMS_EPS = float(np.finfo(np.float32).eps)

    h = x * (1.0 / np.sqrt((x * x).mean(-1, keepdims=True) + RMS_EPS))
    h = h * inputs["g_rms"]
    q = (h @ inputs["Wq"] + inputs["bq"]).reshape(B, T, H, HD).transpose(0, 2, 1, 3)
    k = (h @ inputs["Wk"] + inputs["bk"]).reshape(B, T, H, HD).transpose(0, 2, 1, 3)
    v = (h @ inputs["Wv"]).reshape(B, T, H, HD).transpose(0, 2, 1, 3)

    inv_freq = 1.0 / (10000.0 ** (np.arange(0, HD, 2, dtype=f) / HD))
    ang = np.arange(T, dtype=f)[:, None] * inv_freq[None, :]
    cos, sin = np.cos(ang), np.sin(ang)

    def rope(z):
        z1, z2 = z[..., ::2], z[..., 1::2]
        out = np.stack([z1 * cos - z2 * sin, z1 * sin + z2 * cos], -1)
        return out.reshape(z.shape)

    q, k = rope(q), rope(k)
    scores = np.einsum("bhqd,bhkd->bhqk", q, k) / np.sqrt(np.float32(HD))
    causal = np.tril(np.ones((T, T), bool))
    mask = (np.asarray(inputs["pad_mask"])[:, None, :].astype(bool)
            & causal)[:, None]
    scores = np.where(mask, scores, -np.inf)
    m = scores.max(-1, keepdims=True)
    e = np.exp(scores - m)
    attn = e / e.sum(-1, keepdims=True)
    o = np.einsum("bhqk,bhkd->bhqd", attn, v)
    o = o.transpose(0, 2, 1, 3).reshape(B, T, D)
    x = x + o @ inputs["Wo"] + inputs["bo"]

    mu = x.mean(-1, keepdims=True)
    var = ((x - mu) ** 2).mean(-1, keepdims=True)
    h2 = (x - mu) / np.sqrt(var + 1e-5) * inputs["g_ln"] + inputs["b_ln"]
    u = h2 @ inputs["W1"] + inputs["b1"]
    g1 = u @ inputs["Wg1"] + inputs["bg1"]
    s = (g1 / (1 + np.exp(-g1))) * (u @ inputs["Wg2"] + inputs["bg2"])
    return x + s @ inputs["W2"] + inputs["b2"]


def make_small_inputs(cfg, seed=0):
    B, T, D, H, DFF = cfg["B"], cfg["T"], cfg["D"], cfg["H"], cfg["DFF"]
    rng = np.random.default_rng(seed)
    f = np.float32

    def w(shape, fan):
        return ((rng.random(shape, dtype=f) * 2 - 1) / np.sqrt(fan)).astype(f)

    lengths = rng.integers(T // 2, T + 1, size=(B,))
    pad = (np.arange(T)[None, :] < lengths[:, None]).astype(np.int32)
    return dict(
        x=rng.standard_normal((B, T, D), dtype=f),
        Wq=w((D, D), D), bq=rng.standard_normal(D, dtype=f) * 0.02,
        Wk=w((D, D), D), bk=rng.standard_normal(D, dtype=f) * 0.02,
        Wv=w((D, D), D),
        Wo=w((D, D), D), bo=rng.standard_normal(D, dtype=f) * 0.02,
        W1=w((D, DFF), D), b1=rng.standard_normal(DFF, dtype=f) * 0.02,
        Wg1=w((DFF, DFF), DFF), bg1=rng.standard_normal(DFF, dtype=f) * 0.02,
        Wg2=w((DFF, DFF), DFF), bg2=rng.standard_normal(DFF, dtype=f) * 0.02,
        W2=w((DFF, D), DFF), b2=rng.standard_normal(D, dtype=f) * 0.02,
        g_rms=(1 + 0.1 * rng.standard_normal(D)).astype(f),
        g_ln=(1 + 0.1 * rng.standard_normal(D)).astype(f),
        b_ln=(0.05 * rng.standard_normal(D)).astype(f),
        pad_mask=pad,
    )


# ===================== tile scheduler patch =====================


import concourse.tile as tile


def _split_drain_and_barrier(self, tick_clock, wait_clock):
    from concourse.vector_clock import ScopedClock

    drain_inst = self.nc.sync.drain()
    wait_clock.add_sem_waits(
        drain_inst.ins, ScopedClock({None: tick_clock.global_clock})
    )
    si = drain_inst.ins.sync_info
    waits = list(si.on_wait) if si and si.on_wait else []
    if len(waits) > 1:
        si.on_wait.clear()
        si.on_wait.extend(waits[:1])
        for i in range(1, len(waits), 1):
            extra = self.nc.sync.drain()
            esi = extra.ins.sync_info
            if esi is None:
                import concourse.mybir as mybir

                extra.ins.sync_info = mybir.SyncInfo(
                    on_wait=waits[i : i + 1], on_update=[]
                )
            else:
                esi.on_wait.extend(waits[i : i + 1])

    self.nc.all_engine_barrier()
    assert self.sems is not None
    popped = self.nc._tile_sem_poison_stack.pop()
    assert popped is self._sem_poison
    self.nc.clear_and_free_semaphores(list(self.sems.allocated().values()))
    self.nc.all_engine_barrier()


def split_excess_waits(nc, default_limit=1, ctrl_limit=1, dma_limit=1):
    """Walrus in this container rejects instructions whose sync_info
    carries more wait commands than the ISA encoding has slots for.
    Move excess waits onto same-engine no-op carriers inserted right
    before the offending instruction (engine queues are in-order, so the
    carrier's waits are observed before the instruction issues)."""
    import concourse.mybir as mybir

    CTRL = ("InstDrain", "InstNoOp", "InstEventSemaphore")
    DMA = ("InstDMACopy", "InstTriggeredCopy", "InstDMATranspose")
    nsplit = 0
    for bb_name, bbw in list(nc.bb_map.items()):
        bb = bbw.bb if hasattr(bbw, "bb") else bbw
        insts = bb.instructions
        i = 0
        while i < len(insts):
            inst = insts[i]
            tname = type(inst).__name__
            limit = (ctrl_limit if tname in CTRL
                     else dma_limit if tname in DMA else default_limit)
            si = inst.sync_info
            waits = list(si.on_wait) if si and si.on_wait else []
            if len(waits) > limit:
                keep, extra = waits[:limit], waits[limit:]
                si.on_wait.clear()
                si.on_wait.extend(keep)
                ncar = 0
                for j in range(0, len(extra), ctrl_limit):
                    chunk = extra[j:j + ctrl_limit]
                    car = nc.engines[inst.engine].nop(nofuse=True).ins
                    # nop() appended to the current bb; move it here
                    for other in nc.bb_map.values():
                        obb = other.bb if hasattr(other, "bb") else other
                        if obb.instructions and obb.instructions[-1] is car:
                            obb.instructions.pop()
                            break
                    car.sync_info = mybir.SyncInfo(on_wait=chunk, on_update=[])
                    insts.insert(i, car)
                    ncar += 1
                i += ncar
                nsplit += 1
            i += 1
    return nsplit


def _apply_tile_patch():
    tile.TileContext._drain_and_barrier = _split_drain_and_barrier


# ================================================================ runner

_tile_patch_applied = False
_build_cache = {}
LAST_EXEC_NS = None


def _get_nc():
    global _tile_patch_applied
    if not _tile_patch_applied:
        _apply_tile_patch()
        _tile_patch_applied = True
    if "nc" not in _build_cache:
        nc = bass.Bass()
        build(nc, full_cfg())
        _build_cache["nc"] = nc
    return _build_cache["nc"]


def kernel(_profile=False, **inputs):
    """Full-input decoder block on 8 TRN2 NeuronCores.

    inputs: the arrays from reference.setup_inputs() (numpy or jax).
    Returns the full [B, T, D] float32 output.
    """
    global LAST_EXEC_NS
    from concourse.bass_utils import run_bass_kernel_spmd

    cfg = full_cfg()
    nc = _get_nc()
    in_maps = host_prepare({k: np.asarray(v) for k, v in inputs.items()}, cfg)
    res = run_bass_kernel_spmd(nc, in_maps, list(range(CORES)),
                               trace=bool(_profile))
    LAST_EXEC_NS = getattr(res, "exec_time_ns", None)
    return host_assemble(res.results, cfg)
